# revision 4
# baseline (speedup 1.0000x reference)
"""Trainium2 Bass kernel: modulated deformable conv 3x3 (DCNv2) + BN(eval)
+ ReLU.  B=4, C=O=256, H=W=64, distributed over 8 NeuronCores.

Sharding: core i -> batch b = i//2, image row-half = i%2 (32 rows). Each core
computes out[b, :, h0:h0+32, :] fully (data-parallel over batch x row-half).

Design (v2; ~2.8x over the v1 gather kernel):
  - Bottleneck analysis showed v1 was serialized on Q7 SWDGE descriptor
    generation (582us: 4 descriptors per (tap,sample), 512B each) and on
    per-op DVE overhead in the corner combine.
  - XY2 DRAM scratch [5504 tokens, 512 f16]: token t=(y,x) holds
    [x_pad^T at (y,x) ; x_pad^T at (y+1,x)] (row-pair interleave, 1024B
    token stride).  dma_gather with elem_size=2048B and elem_step=1024B
    (overlapping source AP) then fetches ALL FOUR bilinear corners
    (y0/y1 x x0/x1) with a single descriptor per (tap, sample):
    18432 descriptors total, ~9us Q7 per 896-idx call.
  - Indices: sample math in [sample-part, chunk] layout; the 16x8 bit-swap
    needed by the gather's wrapped int16 index format is folded into the
    PE transpose via a permutation matrix (pm32), so both index DMA hops
    are plain 3D contiguous copies.
  - Corner combine on DVE: weights stored as DUPLICATED adjacent f16
    pairs so the broadcast AP is [nj, 128(stride 0), 2(stride 1)] -- a
    real unit-stride innermost dim, which satisfies the DVE 2x packed
    mode trigger (the usual 0-stride broadcast forces 1x).  Four per-slot
    multiplies at 2x + a 3-add tree per call.
  - V^T->V via PE transposes; main conv accumulates over (tap, c-half) in
    per-chunk PSUM bursts; bias via K=1 matmul; ReLU on ACT.
  - Samples split in 3 blocks of (7, 7, 2) chunks so the final matmul
    burst tail is short; per-block output rows stream out early.
  - BN folded into weights/bias on device; x shipped f16 with x-pad
    pre-applied; host only reorders/transposes layouts.
"""

import numpy as np

import concourse.bass as bass
import concourse.bacc as bacc
import concourse.mybir as mybir
import concourse.tile as tile
from concourse.ap import AP
from concourse import bass_utils

F32 = mybir.dt.float32
F16 = mybir.dt.float16
I16 = mybir.dt.int16
I32 = mybir.dt.int32
AF = mybir.ActivationFunctionType
ALU = mybir.AluOpType

B, C, O, H, W = 4, 256, 256, 64, 64
K = 9
P = 5
W2 = H + 2 * P            # 74
NQ = W2 * W2              # 5476
NQP = 5504                # 43*128
HH = 32                   # rows per core
S = HH * W                # 2048 samples per core
CC = C // 128             # 2
NSB = 2                   # sample blocks
SB = S // NSB             # 1024 samples / block
BLOCKS = [(0, 7), (7, 7), (14, 2)]   # (chunk offset, n chunks)
EPS = 1e-5
N_CORES = 8

_NC_CACHE = {}


def build_nc(dbg=False):
    nc = bacc.Bacc("TRN2", target_bir_lowering=False, debug=False,
                   num_devices=N_CORES, num_swdge_queues=4)

    x_in = nc.dram_tensor("x_b", [C, H, W2], F16, kind="ExternalInput")
    xhalo = nc.dram_tensor("xhalo", [C, 34, W2], F16, kind="ExternalInput")
    w_t = nc.dram_tensor("w_t", [CC, 128, K * O], F32, kind="ExternalInput")
    woff_t = nc.dram_tensor("woff_t", [CC, 128, K * 27], F32, kind="ExternalInput")
    b_off_in = nc.dram_tensor("b_off", [27, 1], F32, kind="ExternalInput")
    bnvec = nc.dram_tensor("bnvec", [1, 5 * O], F32, kind="ExternalInput")
    baseC = nc.dram_tensor("baseC", [128, 16 * 32], F32, kind="ExternalInput")
    ident32 = nc.dram_tensor("ident32", [128, 128], F32, kind="ExternalInput")
    ident16 = nc.dram_tensor("ident16", [128, 128], F16, kind="ExternalInput")
    perm32 = nc.dram_tensor("perm32", [128, 128], F32, kind="ExternalInput")
    ones16 = nc.dram_tensor("ones16", [1, 128], F16, kind="ExternalInput")

    out_d = nc.dram_tensor("out_c", [S, O], F32, kind="ExternalOutput")
    dbg_t = {}
    if dbg:
        dbg_t["om"] = nc.dram_tensor("dbg_om", [27, S], F32, kind="ExternalOutput")
        dbg_t["wr"] = nc.dram_tensor("dbg_wr", [128, K * NSB * 64], I16, kind="ExternalOutput")
        dbg_t["gt"] = nc.dram_tensor("dbg_gt", [128, 8 * 1024], F16, kind="ExternalOutput")
        dbg_t["vt"] = nc.dram_tensor("dbg_vt", [128, 8 * 256], F16, kind="ExternalOutput")
        dbg_t["xy"] = nc.dram_tensor("dbg_xy", [128, 512], F16, kind="ExternalOutput")

    with tile.TileContext(nc) as tc:
        _build(nc, tc, x_in, xhalo, w_t, woff_t, b_off_in, bnvec, baseC,
               ident32, ident16, perm32, ones16, out_d, dbg_t)
    nc.compile()
    return nc


def _build(nc, tc, x_in, xhalo, w_t, woff_t, b_off_in, bnvec, baseC,
           ident32, ident16, perm32, ones16, out_d, dbg_t={}):
    from contextlib import ExitStack

    with ExitStack() as top:
        pers = top.enter_context(tc.tile_pool(name="pers", bufs=1))
        dram = top.enter_context(tc.tile_pool(name="dram", bufs=1, space="DRAM"))
        xy2_t = dram.tile([NQP, 512], F16, name="xy2_scr", tag="xy2")
        idxs_t = dram.tile([K, 16, 128], I16, name="idx_scr", tag="idxs")
        ph1_cm = tc.tile_pool(name="ph1", bufs=1)
        ph1 = ph1_cm.__enter__()

        # ------------- constants -------------
        id32 = pers.tile([128, 128], F32)
        nc.sync.dma_start(out=id32[:], in_=ident32.ap())
        id16 = pers.tile([128, 128], F16)
        nc.sync.dma_start(out=id16[:], in_=ident16.ap())
        pm32 = pers.tile([128, 128], F32)
        nc.sync.dma_start(out=pm32[:], in_=perm32.ap())
        one16 = pers.tile([1, 128], F16)
        nc.sync.dma_start(out=one16[:], in_=ones16.ap())
        base_t = pers.tile([128, 16, 32], F32)
        nc.sync.dma_start(out=base_t[:], in_=baseC.ap().rearrange("p (a b) -> p a b", a=16))
        boff_t = pers.tile([27, 1], F32)
        nc.sync.dma_start(out=boff_t[:], in_=b_off_in.ap())

        # ------------- offset-conv inputs first (om gates the idx chain) ----
        woff16 = []
        for cc in range(CC):
            woff16.append(ph1.tile([128, K * 27], F16, name=f"woff{cc}", tag=f"woff{cc}"))
        with tc.tile_pool(name="wotmp", bufs=1) as wotmp:
            for cc in range(CC):
                wo = wotmp.tile([128, K * 27], F32, name=f"wo{cc}", tag="wo")
                nc.sync.dma_start(out=wo[:], in_=woff_t.ap()[cc])
                nc.vector.tensor_copy(woff16[cc][:], wo[:])
        # loads split into row chunks so dependent compute starts streaming
        xom = []
        xom_rows = [0, 11, 19, 27, 34]
        for cc in range(CC):
            t = ph1.tile([128, 34 * W2], F16, name=f"xom{cc}", tag=f"xom{cc}")
            for a, b in zip(xom_rows[:-1], xom_rows[1:]):
                nc.sync.dma_start(
                    out=t[:, a * W2:b * W2],
                    in_=xhalo.ap()[cc * 128:(cc + 1) * 128, a:b].rearrange(
                        "p h w -> p (h w)"))
            xom.append(t)

        # ------------- xpad (full image, fp16, x-pad pre-applied) -----------
        xpad = []
        for cc in range(CC):
            t = ph1.tile([128, NQP], F16, name=f"xpad{cc}", tag=f"xpad{cc}")
            nc.vector.memset(t[:, 0:P * W2], 0.0)
            nc.vector.memset(t[:, (P + H) * W2:NQP], 0.0)
            for r0 in range(0, H, 16):
                nc.sync.dma_start(
                    out=t[:, (P + r0) * W2:(P + r0 + 16) * W2],
                    in_=x_in.ap()[cc * 128:(cc + 1) * 128, r0:r0 + 16].rearrange(
                        "p h w -> p (h w)"))
            xpad.append(t)

        # ------------- offset conv: om [27, 2048] -------------
        om_sb = ph1.tile([27, S], F32)
        omps_cm = tc.tile_pool(name="omps", bufs=1, space="PSUM")
        omps = omps_cm.__enter__()
        if True:
            om_ps = omps.tile([27, S], F32, name="om_ps", tag="om_ps")
            for bk in range(4):           # 4 banks of 512 samples (8 rows x 64)
                for cc in range(CC):
                    for t9 in range(K):
                        ty, tx = t9 // 3, t9 % 3
                        rhs = xom[cc][:].rearrange("p (h w) -> p h w", w=W2)[
                            :, bk * 8 + ty: bk * 8 + ty + 8,
                            P - 1 + tx: P - 1 + tx + W]
                        nc.tensor.matmul(om_ps[:, bk * 512:(bk + 1) * 512],
                                         woff16[cc][:, t9 * 27:(t9 + 1) * 27], rhs,
                                         start=(cc == 0 and t9 == 0),
                                         stop=(cc == CC - 1 and t9 == K - 1))
            nc.scalar.activation(om_sb[:], om_ps[:], AF.Identity,
                                 bias=boff_t[:, 0:1])
            if "om" in dbg_t:
                nc.sync.dma_start(out=dbg_t["om"].ap(), in_=om_sb[:])

            # ------------- omT [128, 16, 32] (sample-major offsets) ---------
            omT_ps = omps.tile([128, 512], F32, name="omT_ps", tag="omT_ps")
            nc.vector.memset(omT_ps[:], 0.0)
            for ch in range(16):
                nc.tensor.transpose(omT_ps[:, ch * 32: ch * 32 + 27],
                                    om_sb[:, ch * 128:(ch + 1) * 128],
                                    id32[0:27, 0:27])
        omT = omT_ps[:].rearrange("p (a b) -> p a b", a=16)

        # ------------- XY2 build: token t = [XT1[t]; XT1[t+74]] -------------
        # PE-transpose xpad -> token-major chunks, write each chunk twice
        # (cols 0:256 at rows t, cols 256:512 at rows t-74).
        with tc.tile_pool(name="xtp", bufs=3) as xtp, \
             tc.tile_pool(name="xtps", bufs=3, space="PSUM") as xtps:
            for grp in range(11):          # 4 q-chunks per group, 43 chunks
                qcs = list(range(grp * 4, min(grp * 4 + 4, 43)))
                nqc = len(qcs)
                pt = xtps.tile([128, 1024], F16, name=f"xt_ps{grp}", tag="xt_ps")
                for i, qc in enumerate(qcs):
                    for cc in range(CC):
                        nc.tensor.transpose(
                            pt[:, i * 256 + cc * 128: i * 256 + (cc + 1) * 128],
                            xpad[cc][:, qc * 128:(qc + 1) * 128], id16[:])
                st = xtp.tile([128, 1024], F16, name=f"xt_sb{grp}", tag="xt_sb")
                nc.scalar.activation(st[:, 0:nqc * 256], pt[:, 0:nqc * 256], AF.Copy)
                a = grp * 512
                n = nqc * 128
                src3 = st[:, 0:nqc * 256].rearrange("p (qc c) -> p qc c", c=256)
                # write 1: first half of tokens a..a+n
                dst1 = xy2_t[a: a + n, 0:256].rearrange("(qc p) c -> p qc c", p=128)
                nc.sync.dma_start(out=dst1, in_=src3)
                # write 2: second half of tokens a-74..a+n-74
                if grp == 0:
                    # tokens 74..512 -> rows 0..438 (split at partition 74)
                    dst2a = xy2_t[0:54, 256:512]
                    nc.sync.dma_start(
                        out=dst2a, in_=st[74:128, 0:256])
                    dst2b = xy2_t[54: 54 + 384, 256:512].rearrange(
                        "(qc p) c -> p qc c", p=128)
                    nc.sync.dma_start(
                        out=dst2b,
                        in_=st[:, 256:1024].rearrange("p (qc c) -> p qc c", c=256))
                else:
                    dst2 = xy2_t[a - 74: a - 74 + n, 256:512].rearrange(
                        "(qc p) c -> p qc c", p=128)
                    nc.sync.dma_start(out=dst2, in_=src3)

        # ------------- sample math (all [128, 16, .] sample-major) ----------
        ppx = ph1.tile([128, 16, 32], F32)
        nc.vector.tensor_tensor(out=ppx[:], in0=omT, in1=base_t[:], op=ALU.add)
        ii = ph1.tile([128, 16, 18], I32)
        nc.vector.tensor_copy(ii[:], ppx[:, :, 0:18])
        ff = ph1.tile([128, 16, 18], F32)
        nc.vector.tensor_copy(ff[:], ii[:])
        gtt = ph1.tile([128, 16, 18], F32)
        nc.vector.tensor_tensor(out=gtt[:], in0=ff[:], in1=ppx[:, :, 0:18], op=ALU.is_gt)
        flo = ph1.tile([128, 16, 18], F32)
        nc.vector.tensor_tensor(out=flo[:], in0=ff[:], in1=gtt[:], op=ALU.subtract)
        lf = ph1.tile([128, 16, 18], F32)
        nc.vector.tensor_tensor(out=lf[:], in0=ppx[:, :, 0:18], in1=flo[:], op=ALU.subtract)
        floc = ph1.tile([128, 16, 18], F32)
        nc.vector.tensor_scalar(floc[:], flo[:], 0.0, float(W2 - 2), ALU.max, ALU.min)
        msk = ph1.tile([128, 16, 9], F32)
        nc.scalar.activation(msk[:], omT[:, :, 18:27], AF.Sigmoid)
        ol = ph1.tile([128, 16, 18], F32)
        nc.vector.tensor_scalar(ol[:], lf[:], -1.0, 1.0, ALU.mult, ALU.add)
        # corner weights (mask folded) in payload-slot order s = xa*2 + ya,
        # each value DUPLICATED in an adjacent f16 pair: wqs[p, ch, k, s, 0:2].
        # The combine then reads [nj, 128(stride 0), 2(stride 1)] broadcast
        # APs whose innermost dim is real unit-stride, which is what the DVE
        # 2x packed mode checks.
        wqs = pers.tile([128, 16, 9, 4, 2], F16, name="wqs", tag="wqs")
        for s in range(4):
            ya, xa = s % 2, s // 2
            yw = ol if ya == 0 else lf     # (1-ly) or ly
            xw = ol if xa == 0 else lf
            wtmp32 = ph1.tile([128, 16, 9], F32, name=f"wrt_{s}", tag="wrt")
            nc.vector.tensor_tensor(out=wtmp32[:], in0=yw[:, :, 0:9],
                                    in1=xw[:, :, 9:18], op=ALU.mult)
            nc.vector.tensor_tensor(
                out=wqs[:, :, :, s, :],
                in0=wtmp32[:].unsqueeze(3).broadcast_to([128, 16, 9, 2]),
                in1=msk[:].unsqueeze(3).broadcast_to([128, 16, 9, 2]),
                op=ALU.mult)
        # token id t = y0c*74 + x0c  [128, 16, 9]
        tokf = ph1.tile([128, 16, 9], F32)
        nc.vector.tensor_scalar(tokf[:], floc[:, :, 0:9], float(W2), None, ALU.mult)
        nc.vector.tensor_tensor(out=tokf[:], in0=tokf[:], in1=floc[:, :, 9:18], op=ALU.add)
        omps_cm.__exit__(None, None, None)

        # ------------- index build: wrapped [128, 9, 128] i16 ---------------
        # Per block (choff, nj): gather position i = jl*128 + p; wrapped col
        # choff*8 + jl*8 + g must hold tok[p=(g*16+w), ch=choff+jl, k] at
        # partition w (replicated x8).  The 16x8 bit-swap of p is folded into
        # the PE transpose via permutation matrix pm32 (out free index n
        # reads input partition (n%8)*16 + n//8), making both DMA hops plain
        # 3D contiguous copies.
        wrapped_all = pers.tile([128, K, 128], I16, name="wrapped", tag="wrapped")
        with tc.tile_pool(name="idxp", bufs=1) as idxp, \
             tc.tile_pool(name="idxps", bufs=1, space="PSUM") as idxps:
            for bi, (choff, nj) in enumerate(BLOCKS):
                tps = idxps.tile([nj * K, 128], F32, name=f"tk{bi}", tag="tkps")
                nc.tensor.transpose(
                    tps[:],
                    tokf[:, choff:choff + nj, :].rearrange("p a b -> p (a b)"),
                    pm32[:])
                t16 = idxp.tile([nj * K, 128], I16, name=f"t16_{bi}", tag="t16")
                nc.vector.tensor_copy(t16[:], tps[:])
                # hop1: t16[jl*9+k, w*8+g] -> idxs_t[k, w, (choff+jl)*8+g]
                for jl in range(nj):
                    src = t16[jl * K:(jl + 1) * K, :].rearrange(
                        "k (w g) -> k w g", g=8)
                    dst = idxs_t[:, :, (choff + jl) * 8:(choff + jl + 1) * 8]
                    nc.sync.dma_start(out=dst, in_=src)
        # hop2: DRAM -> SBUF; queue-q gathers read idxs via (q+1)*32
        # channels, so replicate across all 128 partitions for queues 0-3
        for rep in range(8):
            nc.sync.dma_start(
                out=wrapped_all[rep * 16:(rep + 1) * 16, :, :],
                in_=idxs_t[:, :].rearrange("k w f -> w k f"))

        # ------------- BN fold -------------
        bn_t = pers.tile([1, 5 * O], F32)
        nc.sync.dma_start(out=bn_t[:], in_=bnvec.ap())
        gam = bn_t[:, 0:O]; bet = bn_t[:, O:2 * O]; rmn = bn_t[:, 2 * O:3 * O]
        rvr = bn_t[:, 3 * O:4 * O]; bia = bn_t[:, 4 * O:5 * O]
        sq = pers.tile([1, O], F32)
        nc.vector.tensor_scalar(sq[:], rvr, float(EPS), None, ALU.add)
        # preload the Sigmoid ACT table so the sample-math chain doesn't
        # stall on ACT_TABLE_LOAD mid-preamble
        sigdum = pers.tile([1, 8], F32)
        nc.scalar.activation(sigdum[:], bn_t[:, 0:8], AF.Sigmoid)
        nc.scalar.activation(sq[:], sq[:], AF.Sqrt)
        sfac = pers.tile([1, O], F32)
        nc.vector.reciprocal(sfac[:], sq[:])
        nc.vector.tensor_tensor(out=sfac[:], in0=sfac[:], in1=gam, op=ALU.mult)
        bpr = pers.tile([1, O], F32)
        nc.vector.tensor_tensor(out=bpr[:], in0=bia, in1=rmn, op=ALU.subtract)
        nc.vector.tensor_tensor(out=bpr[:], in0=bpr[:], in1=sfac[:], op=ALU.mult)
        nc.vector.tensor_tensor(out=bpr[:], in0=bpr[:], in1=bet, op=ALU.add)
        bprow16 = pers.tile([1, O], F16)
        nc.vector.tensor_copy(bprow16[:], bpr[:])
        sbc = pers.tile([128, O], F32)
        ones32 = pers.tile([1, 128], F32)
        nc.vector.memset(ones32[:], 1.0)
        with tc.tile_pool(name="bcp", bufs=1, space="PSUM") as bcp:
            bc_ps = bcp.tile([128, O], F32)
            nc.tensor.matmul(bc_ps[:], ones32[:], sfac[:], start=True, stop=True)
            nc.vector.tensor_copy(sbc[:], bc_ps[:])

        # ------------- main weights (BN-scaled, f16) -- needed only by the
        # first matmul burst, so they load after the om/x inputs.
        wmain = []
        for cc in range(CC):
            wmain.append(pers.tile([128, K * O], F16, name=f"wmain{cc}", tag=f"wmain{cc}"))
        with tc.tile_pool(name="wtmp", bufs=1) as wtmp:
            for cc in range(CC):
                wr = wtmp.tile([128, K * O], F32, name=f"wr{cc}", tag="wr")
                nc.sync.dma_start(out=wr[:], in_=w_t.ap()[cc])
                for k in range(K):
                    nc.vector.tensor_tensor(out=wmain[cc][:, k * O:(k + 1) * O],
                                            in0=wr[:, k * O:(k + 1) * O],
                                            in1=sbc[:], op=ALU.mult)

        if "wr" in dbg_t:
            nc.sync.dma_start(out=dbg_t["wr"].ap(),
                              in_=wrapped_all[:].rearrange("p a b -> p (a b)"))
        if "xy" in dbg_t:
            xyrd = pers.tile([128, 512], F16, name="xyrd", tag="xyrd")
            nc.sync.dma_start(out=xyrd[:], in_=xy2_t[1000:1128, :])
            nc.sync.dma_start(out=dbg_t["xy"].ap(), in_=xyrd[:])

        # ------------- main loop -------------
        ph1_cm.__exit__(None, None, None)
        outT = pers.tile([128, 16, O], F32, name="outT", tag="outT")
        gsrc = AP(xy2_t.tensor, 0, [(512, NQP - 1), (1, 1024)])

        with tc.tile_pool(name="mg", bufs=3) as mg, \
             tc.tile_pool(name="mv", bufs=2) as mv, \
             tc.tile_pool(name="mps", bufs=2, space="PSUM") as mps, \
             tc.tile_pool(name="accp", bufs=3, space="PSUM") as accp:
            gq = 0
            for bi, (choff, nj) in enumerate(BLOCKS):
                with tc.tile_pool(name=f"mvs{bi}", bufs=1) as mvs:
                    vsbs = []
                    for k in range(K):
                        gt = mg.tile([128, nj, 1024], F16, name=f"g{bi}_{k}",
                                     tag="gt")
                        nc.gpsimd.dma_gather(
                            gt[:], gsrc,
                            wrapped_all[:, k, choff * 8:(choff + nj) * 8],
                            nj * 128, nj * 128, 1024,
                            elem_step=512, single_packet=False,
                            queue_num=gq % 4)
                        gq += 1
                        # combine: 4 per-slot mults (2x-eligible APs:
                        # innermost real stride-1 dup pair) + a 3-add tree
                        m4 = mv.tile([128, nj, 4, 256], F16,
                                     name=f"m4_{bi}_{k}", tag="m4")
                        vt = mv.tile([128, nj, 256], F16, name=f"v{bi}_{k}",
                                     tag="vt")
                        for sl in range(4):
                            g_sl = gt[:, :, sl * 256:(sl + 1) * 256].rearrange(
                                "p j (r d) -> p j r d", d=2)
                            w_sl = (wqs[:, choff:choff + nj, k, sl, :]
                                    .unsqueeze(2).broadcast_to([128, nj, 128, 2]))
                            o_sl = m4[:, :, sl, :].rearrange(
                                "p j (r d) -> p j r d", d=2)
                            nc.vector.tensor_tensor(out=o_sl, in0=g_sl,
                                                    in1=w_sl, op=ALU.mult)
                        nc.vector.tensor_tensor(out=vt[:], in0=m4[:, :, 0, :],
                                                in1=m4[:, :, 1, :], op=ALU.add)
                        nc.vector.tensor_tensor(out=vt[:], in0=vt[:],
                                                in1=m4[:, :, 2, :], op=ALU.add)
                        nc.vector.tensor_tensor(out=vt[:], in0=vt[:],
                                                in1=m4[:, :, 3, :], op=ALU.add)
                        # transpose V^T[s,c] -> V[c,s] (nj*2 x [128,128])
                        vps = mps.tile([128, nj * 2 * 128], F16,
                                       name=f"vps{bi}_{k}", tag="vps")
                        for j in range(nj):
                            for cc in range(CC):
                                nc.tensor.transpose(
                                    vps[:, (cc * nj + j) * 128:(cc * nj + j + 1) * 128],
                                    vt[:, j, cc * 128:(cc + 1) * 128], id16[:])
                        vsb = mvs.tile([128, nj * 2 * 128], F16,
                                       name=f"vs{bi}_{k}", tag=f"vsb{k}")
                        nc.scalar.activation(vsb[:], vps[:], AF.Copy)
                        vsbs.append(vsb)
                    # per-chunk matmul bursts: own PSUM tile per chunk,
                    # accumulate over (k, cc) within one group
                    for j in range(nj):
                        acc = accp.tile([128, O], F32, name=f"acc{bi}_{j}",
                                        tag="acc")
                        for k in range(K):
                            for cc in range(CC):
                                nc.tensor.matmul(
                                    acc[:],
                                    vsbs[k][:, (cc * nj + j) * 128:(cc * nj + j + 1) * 128],
                                    wmain[cc][:, k * O:(k + 1) * O],
                                    start=(k == 0 and cc == 0),
                                    stop=False)
                        nc.tensor.matmul(acc[:], one16[:], bprow16[:],
                                         start=False, stop=True)
                        nc.scalar.activation(
                            outT[:, choff + j, :], acc[:], AF.Relu)
                # stream this block's output rows out now
                nc.sync.dma_start(
                    out=out_d.ap()[choff * 128:(choff + nj) * 128, :].rearrange(
                        "(ch p) o -> p ch o", p=128),
                    in_=outT[:, choff:choff + nj, :])


# ===================== host side =====================

def _host_prep(inputs):
    """Build the 8 per-core input maps (layout-only host work + constants)."""
    x = np.ascontiguousarray(inputs["x"], dtype=np.float32)
    w_off = np.asarray(inputs["w_off"], np.float32)
    b_off = np.asarray(inputs["b_off"], np.float32)
    weight = np.asarray(inputs["weight"], np.float32)
    bias = np.asarray(inputs["bias"], np.float32)
    gamma = np.asarray(inputs["gamma"], np.float32)
    beta = np.asarray(inputs["beta"], np.float32)
    run_mean = np.asarray(inputs["run_mean"], np.float32)
    run_var = np.asarray(inputs["run_var"], np.float32)

    # weight [O, C, 3, 3] -> [CC, 128c, K, O] -> [CC, 128, K*O]
    wt = weight.reshape(O, C, K).transpose(1, 2, 0).reshape(CC, 128, K * O)
    wt = np.ascontiguousarray(wt)
    wofft = w_off.reshape(27, C, K).transpose(1, 2, 0).reshape(CC, 128, K * 27)
    wofft = np.ascontiguousarray(wofft)
    bnv = np.concatenate([gamma, beta, run_mean, run_var, bias]).astype(np.float32).reshape(1, 5 * O)
    id32 = np.eye(128, dtype=np.float32)
    id16 = np.eye(128, dtype=np.float16)
    perm = np.zeros((128, 128), np.float32)
    for n in range(128):
        perm[(n % 8) * 16 + n // 8, n] = 1.0
    ones = np.ones((1, 128), np.float16)
    boff = b_off.reshape(27, 1).astype(np.float32)

    # x as f16 with x-direction pad pre-applied: [B, C, H, W2]
    xp16 = np.zeros((B, C, H, W2), np.float16)
    xp16[:, :, :, P:P + W] = x.astype(np.float16)

    in_maps = []
    for core in range(N_CORES):
        b, half = core // 2, core % 2
        h0 = half * HH
        # halo rows [h0-1, h0+33) with zero pad at the image boundary
        halo = np.zeros((C, 34, W2), np.float16)
        lo, hi = h0 - 1, h0 + 33
        slo, shi = max(lo, 0), min(hi, H)
        halo[:, slo - lo: slo - lo + (shi - slo)] = xp16[b, :, slo:shi]
        # baseC [128, 16, 32]: cols 0-8 pyP base, 9-17 pxP base, rest 0
        basec = np.zeros((128, 16, 32), np.float32)
        pp_ = np.arange(128)
        for ch in range(16):
            s_ = ch * 128 + pp_
            hloc = h0 + s_ // W
            wloc = s_ % W
            for k in range(K):
                basec[:, ch, k] = hloc + (k // 3) - 1 + P
                basec[:, ch, 9 + k] = wloc + (k % 3) - 1 + P
        in_maps.append({
            "x_b": np.ascontiguousarray(xp16[b]),
            "xhalo": halo,
            "w_t": wt,
            "woff_t": wofft,
            "b_off": boff,
            "bnvec": bnv,
            "baseC": basec.reshape(128, 16 * 32),
            "ident32": id32,
            "ident16": id16,
            "perm32": perm,
            "ones16": ones,
        })
    return in_maps


def _get_nc():
    if "nc" not in _NC_CACHE:
        _NC_CACHE["nc"] = build_nc()
    return _NC_CACHE["nc"]


def kernel(**inputs):
    nc = _get_nc()
    in_maps = _host_prep(inputs)
    res = bass_utils.run_bass_kernel_spmd(nc, in_maps, core_ids=list(range(N_CORES)))
    out = np.zeros((B, O, H, W), np.float32)
    for core in range(N_CORES):
        b, half = core // 2, core % 2
        oc = res.results[core]["out_c"].reshape(16, 128, O)
        arr = oc.reshape(S, O)            # s = ch*128 + p ordering
        out[b, :, half * HH:(half + 1) * HH, :] = (
            arr.reshape(HH, W, O).transpose(2, 0, 1))
    return out



# revision 11
# speedup vs baseline: 1.0706x; 1.0706x over previous
"""Trainium2 Bass kernel: modulated deformable conv 3x3 (DCNv2) + BN(eval)
+ ReLU.  B=4, C=O=256, H=W=64, distributed over 8 NeuronCores.

Sharding: core i -> batch b = i//2, image row-half = i%2 (32 rows). Each core
computes out[b, :, h0:h0+32, :] fully (data-parallel over batch x row-half).

Design v4 (from v2 @316us; bottleneck = Q7 SWDGE descriptor gen ~10ns/idx,
which is serial on the Pool engine regardless of queue_num — measured):
  - Per-core 41-row STRIP input (h0-aligned, zero-padded) instead of full
    image + separate halo: om conv reads the strip directly (xhalo input
    and its 1.2MB load die), XY2 scratch shrinks 5504->3072 tokens
    (24 transposes-chunks instead of 43, -2.5MB writes).
  - Preamble split by sample-half: om banks 0-1 -> omT/math/idx for half 0
    -> first gathers at ~30us (vs 60); om banks 2-3 + half-1 chain overlap
    the first gathers.
  - 18 gather calls (9 taps x 2 halves, 1024 idxs each) instead of 27
    smaller ones: Q7 fixed cost amortized, fewer sem waits/drains.
  - Main loop per (half, k): gather [128,8,1024] -> 4 mults + 3 adds (DVE,
    dup-pair 2x APs) -> 16 PE transposes V^T->V -> ACT copy -> 16
    accumulating matmuls into per-chunk-pair PSUM acc tiles; bias via
    K=1 matmul; ReLU on ACT; per-half output DMA.
  - XY2 DRAM scratch [3072 tokens, 512 f16]: token t=(l,x) holds
    [strip^T at (l,x) ; strip^T at (l+1,x)] (row-pair interleave, 1024B
    stride). dma_gather elem_size=2048B / elem_step=1024B fetches all 4
    bilinear corners with one descriptor per (tap, sample).
  - Indices: 16x8 bit-swap folded into PE transpose via perm matrix pm32;
    both idx DMA hops are plain 3D contiguous copies.
"""

import numpy as np

import concourse.bass as bass
import concourse.bacc as bacc
import concourse.mybir as mybir
import concourse.tile as tile
from concourse.ap import AP
from concourse import bass_utils

F32 = mybir.dt.float32
F16 = mybir.dt.float16
I16 = mybir.dt.int16
I32 = mybir.dt.int32
AF = mybir.ActivationFunctionType
ALU = mybir.AluOpType

B, C, O, H, W = 4, 256, 256, 64, 64
K = 9
P = 5                     # x-pad (cols) and strip top margin
W2 = W + 2 * P            # 74
SR = 41                   # strip rows per core (h0-aligned, padded coords)
NT = 40 * W2              # 2960 usable tokens (l in [0,40))
NQP = 3072                # 24*128 padded token count
HH = 32                   # output rows per core
S = HH * W                # 2048 samples per core
CC = C // 128             # 2
NH = 2                    # sample halves
SH = S // NH              # 1024 samples per half (8 chunks of 128)
JH = 8                    # chunks per half
EPS = 1e-5
N_CORES = 8

_NC_CACHE = {}


def build_nc():
    nc = bacc.Bacc("TRN2", target_bir_lowering=False, debug=False,
                   num_devices=N_CORES)

    xs_in = nc.dram_tensor("xstrip", [C, SR, W2], F16, kind="ExternalInput")
    w_t = nc.dram_tensor("w_t", [CC, 128, K * O], F32, kind="ExternalInput")
    woff_t = nc.dram_tensor("woff_t", [CC, 128, K * 27], F32, kind="ExternalInput")
    b_off_in = nc.dram_tensor("b_off", [27, 1], F32, kind="ExternalInput")
    bnvec = nc.dram_tensor("bnvec", [1, 5 * O], F32, kind="ExternalInput")
    baseC = nc.dram_tensor("baseC", [128, 16 * 32], F32, kind="ExternalInput")
    ident32 = nc.dram_tensor("ident32", [128, 128], F32, kind="ExternalInput")
    ident16 = nc.dram_tensor("ident16", [128, 128], F16, kind="ExternalInput")
    perm32 = nc.dram_tensor("perm32", [128, 128], F32, kind="ExternalInput")
    ones16 = nc.dram_tensor("ones16", [1, 128], F16, kind="ExternalInput")

    out_d = nc.dram_tensor("out_c", [S, O], F32, kind="ExternalOutput")

    with tile.TileContext(nc) as tc:
        _build(nc, tc, xs_in, w_t, woff_t, b_off_in, bnvec, baseC,
               ident32, ident16, perm32, ones16, out_d)
    nc.compile()
    return nc


def _build(nc, tc, xs_in, w_t, woff_t, b_off_in, bnvec, baseC,
           ident32, ident16, perm32, ones16, out_d):
    from contextlib import ExitStack

    with ExitStack() as top:
        pers = top.enter_context(tc.tile_pool(name="pers", bufs=1))
        dram = top.enter_context(tc.tile_pool(name="dram", bufs=1, space="DRAM"))
        xy2_t = dram.tile([NQP, 512], F16, name="xy2_scr", tag="xy2")
        idxs_t = dram.tile([K, 16, 128], I16, name="idx_scr", tag="idxs")
        ph1_cm = tc.tile_pool(name="ph1", bufs=1)
        ph1 = ph1_cm.__enter__()

        # ------------- constants -------------
        id32 = pers.tile([128, 128], F32)
        nc.sync.dma_start(out=id32[:], in_=ident32.ap())
        id16 = pers.tile([128, 128], F16)
        nc.sync.dma_start(out=id16[:], in_=ident16.ap())
        pm32 = pers.tile([128, 128], F32)
        nc.sync.dma_start(out=pm32[:], in_=perm32.ap())
        one16 = pers.tile([1, 128], F16)
        nc.sync.dma_start(out=one16[:], in_=ones16.ap())
        base_t = pers.tile([128, 16, 32], F32)
        nc.sync.dma_start(out=base_t[:], in_=baseC.ap().rearrange("p (a b) -> p a b", a=16))
        boff_t = pers.tile([27, 1], F32)
        nc.sync.dma_start(out=boff_t[:], in_=b_off_in.ap())

        # ------------- offset-conv weights (gate the om chain) --------------
        woff16 = []
        for cc in range(CC):
            woff16.append(ph1.tile([128, K * 27], F16, name=f"woff{cc}", tag=f"woff{cc}"))
        with tc.tile_pool(name="wotmp", bufs=1) as wotmp:
            for cc in range(CC):
                wo = wotmp.tile([128, K * 27], F32, name=f"wo{cc}", tag="wo")
                nc.sync.dma_start(out=wo[:], in_=woff_t.ap()[cc])
                nc.vector.tensor_copy(woff16[cc][:], wo[:])

        # ------------- strip load (f16, both pads pre-applied) --------------
        # row-chunk loads so dependent compute starts streaming; om bank bk
        # needs strip rows 4+8bk .. 13+8bk, so load in 4 chunks of ~11 rows.
        xst = []
        xs_rows = [0, 11, 21, 31, 41]
        for cc in range(CC):
            t = ph1.tile([128, NQP], F16, name=f"xst{cc}", tag=f"xst{cc}")
            nc.vector.memset(t[:, SR * W2:NQP], 0.0)
            for a, b in zip(xs_rows[:-1], xs_rows[1:]):
                nc.sync.dma_start(
                    out=t[:, a * W2:b * W2],
                    in_=xs_in.ap()[cc * 128:(cc + 1) * 128, a:b].rearrange(
                        "p h w -> p (h w)"))
            xst.append(t)

        # ------------- offset conv: om [27, 2048], banks of 512 -------------
        # bank bk covers local rows 8bk..8bk+8; rhs rows (strip) 4+8bk+ty.
        om_sb = ph1.tile([27, S], F32)
        omps_cm = tc.tile_pool(name="omps", bufs=1, space="PSUM")
        omps = omps_cm.__enter__()
        om_ps = omps.tile([27, S], F32, name="om_ps", tag="om_ps")
        for bk in range(4):
            for cc in range(CC):
                for t9 in range(K):
                    ty, tx = t9 // 3, t9 % 3
                    rhs = xst[cc][:, 0:SR * W2].rearrange("p (h w) -> p h w", w=W2)[
                        :, bk * 8 + 4 + ty: bk * 8 + 4 + ty + 8,
                        P - 1 + tx: P - 1 + tx + W]
                    nc.tensor.matmul(om_ps[:, bk * 512:(bk + 1) * 512],
                                     woff16[cc][:, t9 * 27:(t9 + 1) * 27], rhs,
                                     start=(cc == 0 and t9 == 0),
                                     stop=(cc == CC - 1 and t9 == K - 1))
        # per-half copies so half-0 math starts while banks 2-3 still run
        for h in range(NH):
            nc.scalar.activation(om_sb[:, h * SH:(h + 1) * SH],
                                 om_ps[:, h * SH:(h + 1) * SH], AF.Identity,
                                 bias=boff_t[:, 0:1])

        # ------------- XY2 build: token t = [XT[t]; XT[t+74]] ---------------
        with tc.tile_pool(name="xtp", bufs=3) as xtp, \
             tc.tile_pool(name="xtps", bufs=3, space="PSUM") as xtps:
            for grp in range(6):           # 4 q-chunks per group, 24 chunks
                qcs = list(range(grp * 4, grp * 4 + 4))
                pt = xtps.tile([128, 1024], F16, name=f"xt_ps{grp}", tag="xt_ps")
                for i, qc in enumerate(qcs):
                    for cc in range(CC):
                        nc.tensor.transpose(
                            pt[:, i * 256 + cc * 128: i * 256 + (cc + 1) * 128],
                            xst[cc][:, qc * 128:(qc + 1) * 128], id16[:])
                st = xtp.tile([128, 1024], F16, name=f"xt_sb{grp}", tag="xt_sb")
                nc.scalar.activation(st[:], pt[:], AF.Copy)
                a = grp * 512
                src3 = st[:].rearrange("p (qc c) -> p qc c", c=256)
                # write 1: first half of tokens a..a+512
                dst1 = xy2_t[a: a + 512, 0:256].rearrange("(qc p) c -> p qc c", p=128)
                nc.sync.dma_start(out=dst1, in_=src3)
                # write 2: second half of tokens a-74..a+512-74
                if grp == 0:
                    dst2a = xy2_t[0:54, 256:512]
                    nc.sync.dma_start(out=dst2a, in_=st[74:128, 0:256])
                    dst2b = xy2_t[54: 54 + 384, 256:512].rearrange(
                        "(qc p) c -> p qc c", p=128)
                    nc.sync.dma_start(
                        out=dst2b,
                        in_=st[:, 256:1024].rearrange("p (qc c) -> p qc c", c=256))
                else:
                    dst2 = xy2_t[a - 74: a - 74 + 512, 256:512].rearrange(
                        "(qc p) c -> p qc c", p=128)
                    nc.sync.dma_start(out=dst2, in_=src3)

        # ------------- per-half: omT, sample math, weights, indices ---------
        # all sample-major [128, 8, .]; wqs/wrapped persist for the main loop
        wqs = pers.tile([128, 16, 9, 4, 2], F16, name="wqs", tag="wqs")
        wrapped_all = pers.tile([128, K, 128], I16, name="wrapped", tag="wrapped")
        omT_all = ph1.tile([128, 16, 32], F32, name="omT_all", tag="omT_all")

        with tc.tile_pool(name="mth", bufs=2) as mth, \
             tc.tile_pool(name="mthps", bufs=2, space="PSUM") as mthps:
            for h in range(NH):
                c0 = h * JH
                omT_ps = mthps.tile([128, 256], F32, name=f"omT{h}", tag="omT_ps")
                nc.vector.memset(omT_ps[:], 0.0)
                for ch in range(JH):
                    nc.tensor.transpose(
                        omT_ps[:, ch * 32: ch * 32 + 27],
                        om_sb[:, (c0 + ch) * 128:(c0 + ch + 1) * 128],
                        id32[0:27, 0:27])
                omT = omT_ps[:].rearrange("p (a b) -> p a b", a=JH)
                nc.vector.tensor_tensor(out=omT_all[:, c0:c0 + JH, :], in0=omT,
                                        in1=base_t[:, c0:c0 + JH, :], op=ALU.add)
                ppx = omT_all[:, c0:c0 + JH, :]
                ii = mth.tile([128, JH, 18], I32, name=f"ii{h}", tag="ii")
                nc.vector.tensor_copy(ii[:], ppx[:, :, 0:18])
                ff = mth.tile([128, JH, 18], F32, name=f"ff{h}", tag="ff")
                nc.vector.tensor_copy(ff[:], ii[:])
                gtt = mth.tile([128, JH, 18], F32, name=f"gtt{h}", tag="gtt")
                nc.vector.tensor_tensor(out=gtt[:], in0=ff[:], in1=ppx[:, :, 0:18],
                                        op=ALU.is_gt)
                flo = mth.tile([128, JH, 18], F32, name=f"flo{h}", tag="flo")
                nc.vector.tensor_tensor(out=flo[:], in0=ff[:], in1=gtt[:],
                                        op=ALU.subtract)
                lf = mth.tile([128, JH, 18], F32, name=f"lf{h}", tag="lf")
                nc.vector.tensor_tensor(out=lf[:], in0=ppx[:, :, 0:18], in1=flo[:],
                                        op=ALU.subtract)
                floc = mth.tile([128, JH, 18], F32, name=f"floc{h}", tag="floc")
                # clip y to [0, 39], x to [0, 72]: columns 0:9 are y, 9:18 x
                nc.vector.tensor_scalar(floc[:, :, 0:9], flo[:, :, 0:9],
                                        0.0, 39.0, ALU.max, ALU.min)
                nc.vector.tensor_scalar(floc[:, :, 9:18], flo[:, :, 9:18],
                                        0.0, float(W2 - 2), ALU.max, ALU.min)
                msk = mth.tile([128, JH, 9], F32, name=f"msk{h}", tag="msk")
                nc.scalar.activation(msk[:], omT[:, :, 18:27], AF.Sigmoid)
                ol = mth.tile([128, JH, 18], F32, name=f"ol{h}", tag="ol")
                nc.vector.tensor_scalar(ol[:], lf[:], -1.0, 1.0, ALU.mult, ALU.add)
                # corner weights (mask folded), slot s = xa*2 + ya, DUPLICATED
                # adjacent f16 pairs for the DVE 2x packed-mode combine APs
                for sq in range(4):
                    ya, xa = sq % 2, sq // 2
                    yw = ol if ya == 0 else lf
                    xw = ol if xa == 0 else lf
                    wtmp32 = mth.tile([128, JH, 9], F32, name=f"wrt{h}_{sq}", tag="wrt")
                    nc.vector.tensor_tensor(out=wtmp32[:], in0=yw[:, :, 0:9],
                                            in1=xw[:, :, 9:18], op=ALU.mult)
                    nc.vector.tensor_tensor(
                        out=wqs[:, c0:c0 + JH, :, sq, :],
                        in0=wtmp32[:].unsqueeze(3).broadcast_to([128, JH, 9, 2]),
                        in1=msk[:].unsqueeze(3).broadcast_to([128, JH, 9, 2]),
                        op=ALU.mult)
                # token id t = y0c*74 + x0c  [128, 8, 9]
                tokf = mth.tile([128, JH, 9], F32, name=f"tokf{h}", tag="tokf")
                nc.vector.tensor_scalar(tokf[:], floc[:, :, 0:9], float(W2), None,
                                        ALU.mult)
                nc.vector.tensor_tensor(out=tokf[:], in0=tokf[:],
                                        in1=floc[:, :, 9:18], op=ALU.add)
                # index build: PE transpose with perm matrix folds the 16x8
                # bit-swap; then two plain DMA hops to wrapped layout
                tps = mthps.tile([JH * K, 128], F32, name=f"tk{h}", tag="tkps")
                nc.tensor.transpose(
                    tps[:], tokf[:].rearrange("p a b -> p (a b)"), pm32[:])
                t16 = mth.tile([JH * K, 128], I16, name=f"t16_{h}", tag="t16")
                nc.vector.tensor_copy(t16[:], tps[:])
                for jl in range(JH):
                    src = t16[jl * K:(jl + 1) * K, :].rearrange(
                        "k (w g) -> k w g", g=8)
                    dst = idxs_t[:, :, (c0 + jl) * 8:(c0 + jl + 1) * 8]
                    nc.sync.dma_start(out=dst, in_=src)
                # hop2: queue-0 gather reads idxs via 32 channels -> 2 replicas
                for rep in range(2):
                    nc.sync.dma_start(
                        out=wrapped_all[rep * 16:(rep + 1) * 16, :,
                                        h * 64:(h + 1) * 64],
                        in_=idxs_t[:, :, h * 64:(h + 1) * 64].rearrange(
                            "k w f -> w k f"))

        omps_cm.__exit__(None, None, None)

        # ------------- BN fold -------------
        bn_t = pers.tile([1, 5 * O], F32)
        nc.sync.dma_start(out=bn_t[:], in_=bnvec.ap())
        gam = bn_t[:, 0:O]; bet = bn_t[:, O:2 * O]; rmn = bn_t[:, 2 * O:3 * O]
        rvr = bn_t[:, 3 * O:4 * O]; bia = bn_t[:, 4 * O:5 * O]
        sq = pers.tile([1, O], F32)
        nc.vector.tensor_scalar(sq[:], rvr, float(EPS), None, ALU.add)
        # preload the Sigmoid ACT table early so the math chain doesn't stall
        sigdum = pers.tile([1, 8], F32)
        nc.scalar.activation(sigdum[:], bn_t[:, 0:8], AF.Sigmoid)
        nc.scalar.activation(sq[:], sq[:], AF.Sqrt)
        sfac = pers.tile([1, O], F32)
        nc.vector.reciprocal(sfac[:], sq[:])
        nc.vector.tensor_tensor(out=sfac[:], in0=sfac[:], in1=gam, op=ALU.mult)
        bpr = pers.tile([1, O], F32)
        nc.vector.tensor_tensor(out=bpr[:], in0=bia, in1=rmn, op=ALU.subtract)
        nc.vector.tensor_tensor(out=bpr[:], in0=bpr[:], in1=sfac[:], op=ALU.mult)
        nc.vector.tensor_tensor(out=bpr[:], in0=bpr[:], in1=bet, op=ALU.add)
        bprow16 = pers.tile([1, O], F16)
        nc.vector.tensor_copy(bprow16[:], bpr[:])
        # bias duplicated [1, 2*O] so one start=True matmul initializes a
        # whole chunk-pair PSUM bank (bank-wide clear happens exactly once)
        bprow16d = pers.tile([1, 2 * O], F16)
        nc.vector.tensor_copy(bprow16d[:, 0:O], bpr[:])
        nc.vector.tensor_copy(bprow16d[:, O:2 * O], bpr[:])
        sbc = pers.tile([128, O], F32)
        ones32 = pers.tile([1, 128], F32)
        nc.vector.memset(ones32[:], 1.0)
        with tc.tile_pool(name="bcp", bufs=1, space="PSUM") as bcp:
            bc_ps = bcp.tile([128, O], F32)
            nc.tensor.matmul(bc_ps[:], ones32[:], sfac[:], start=True, stop=True)
            nc.vector.tensor_copy(sbc[:], bc_ps[:])

        # ------------- main weights (BN-scaled, f16) -------------
        wmain = []
        for cc in range(CC):
            wmain.append(pers.tile([128, K * O], F16, name=f"wmain{cc}", tag=f"wmain{cc}"))
        with tc.tile_pool(name="wtmp", bufs=1) as wtmp:
            for cc in range(CC):
                wr = wtmp.tile([128, K * O], F32, name=f"wr{cc}", tag="wr")
                nc.sync.dma_start(out=wr[:], in_=w_t.ap()[cc])
                for k in range(K):
                    nc.vector.tensor_tensor(out=wmain[cc][:, k * O:(k + 1) * O],
                                            in0=wr[:, k * O:(k + 1) * O],
                                            in1=sbc[:], op=ALU.mult)

        # ------------- main loop: per (half, tap) -------------
        ph1_cm.__exit__(None, None, None)
        gsrc = AP(xy2_t.tensor, 0, [(512, NQP - 1), (1, 1024)])

        with tc.tile_pool(name="mg", bufs=2) as mg, \
             tc.tile_pool(name="mv", bufs=2) as mv, \
             tc.tile_pool(name="mvs", bufs=3) as mvs, \
             tc.tile_pool(name="mo", bufs=2) as mo, \
             tc.tile_pool(name="mps", bufs=1, space="PSUM") as mps, \
             tc.tile_pool(name="accp", bufs=1, space="PSUM") as accp:
            for h in range(NH):
                c0 = h * JH
                accs = [accp.tile([128, 512], F32, name=f"acc{h}_{pj}",
                                  tag=f"acc{pj}") for pj in range(4)]
                # bias-init each chunk-pair bank (start=True exactly once per
                # bank; all tap matmuls then accumulate with start=False)
                for pj in range(4):
                    nc.tensor.matmul(accs[pj][:], one16[:], bprow16d[:],
                                     start=True, stop=False)
                outT = mo.tile([128, JH, O], F32, name=f"outT{h}", tag="outT")
                for k in range(K):
                    gt = mg.tile([128, JH, 1024], F16, name=f"g{h}_{k}", tag="gt")
                    nc.gpsimd.dma_gather(
                        gt[:], gsrc,
                        wrapped_all[:, k, h * 64:(h + 1) * 64],
                        SH, SH, 1024,
                        elem_step=512, single_packet=False)
                    # combine: 4 per-slot mults (2x-eligible dup-pair APs)
                    # + a 3-add tree
                    m4 = mv.tile([128, JH, 4, 256], F16, name=f"m4_{h}_{k}",
                                 tag="m4")
                    vt = mv.tile([128, JH, 256], F16, name=f"v{h}_{k}", tag="vt")
                    for sl in range(4):
                        g_sl = gt[:, :, sl * 256:(sl + 1) * 256].rearrange(
                            "p j (r d) -> p j r d", d=2)
                        w_sl = (wqs[:, c0:c0 + JH, k, sl, :]
                                .unsqueeze(2).broadcast_to([128, JH, 128, 2]))
                        o_sl = m4[:, :, sl, :].rearrange(
                            "p j (r d) -> p j r d", d=2)
                        nc.vector.tensor_tensor(out=o_sl, in0=g_sl,
                                                in1=w_sl, op=ALU.mult)
                    nc.vector.tensor_tensor(out=vt[:], in0=m4[:, :, 0, :],
                                            in1=m4[:, :, 1, :], op=ALU.add)
                    nc.vector.tensor_tensor(out=vt[:], in0=vt[:],
                                            in1=m4[:, :, 2, :], op=ALU.add)
                    nc.vector.tensor_tensor(out=vt[:], in0=vt[:],
                                            in1=m4[:, :, 3, :], op=ALU.add)
                    # transpose V^T[s,c] -> V[c,s] (JH*2 x [128,128])
                    vps = mps.tile([128, JH * 2 * 128], F16, name=f"vps{h}_{k}",
                                   tag="vps")
                    for j in range(JH):
                        for cc in range(CC):
                            nc.tensor.transpose(
                                vps[:, (cc * JH + j) * 128:(cc * JH + j + 1) * 128],
                                vt[:, j, cc * 128:(cc + 1) * 128], id16[:])
                    vsb = mvs.tile([128, JH * 2 * 128], F16, name=f"vs{h}_{k}",
                                   tag="vsb")
                    nc.scalar.activation(vsb[:], vps[:], AF.Copy)
                    # accumulate this tap into the 8 per-chunk acc slices
                    for j in range(JH):
                        acc = accs[j // 2][:, (j % 2) * 256:(j % 2) * 256 + 256]
                        for cc in range(CC):
                            nc.tensor.matmul(
                                acc,
                                vsb[:, (cc * JH + j) * 128:(cc * JH + j + 1) * 128],
                                wmain[cc][:, k * O:(k + 1) * O],
                                start=False,
                                stop=(k == K - 1 and cc == CC - 1 and j % 2 == 1))
                # one ReLU per bank after both its chunks finish
                for pj in range(4):
                    nc.scalar.activation(
                        outT[:].rearrange("p a b -> p (a b)")[
                            :, pj * 512:(pj + 1) * 512],
                        accs[pj][:], AF.Relu)
                # stream this half's output rows out
                nc.sync.dma_start(
                    out=out_d.ap()[c0 * 128:(c0 + JH) * 128, :].rearrange(
                        "(ch p) o -> p ch o", p=128),
                    in_=outT[:])


# ===================== host side =====================

def _host_prep(inputs):
    """Build the 8 per-core input maps (layout-only host work + constants)."""
    x = np.ascontiguousarray(inputs["x"], dtype=np.float32)
    w_off = np.asarray(inputs["w_off"], np.float32)
    b_off = np.asarray(inputs["b_off"], np.float32)
    weight = np.asarray(inputs["weight"], np.float32)
    bias = np.asarray(inputs["bias"], np.float32)
    gamma = np.asarray(inputs["gamma"], np.float32)
    beta = np.asarray(inputs["beta"], np.float32)
    run_mean = np.asarray(inputs["run_mean"], np.float32)
    run_var = np.asarray(inputs["run_var"], np.float32)

    # weight [O, C, 3, 3] -> [CC, 128c, K, O] -> [CC, 128, K*O]
    wt = weight.reshape(O, C, K).transpose(1, 2, 0).reshape(CC, 128, K * O)
    wt = np.ascontiguousarray(wt)
    wofft = w_off.reshape(27, C, K).transpose(1, 2, 0).reshape(CC, 128, K * 27)
    wofft = np.ascontiguousarray(wofft)
    bnv = np.concatenate([gamma, beta, run_mean, run_var, bias]).astype(np.float32).reshape(1, 5 * O)
    id32 = np.eye(128, dtype=np.float32)
    id16 = np.eye(128, dtype=np.float16)
    perm = np.zeros((128, 128), np.float32)
    for n in range(128):
        perm[(n % 8) * 16 + n // 8, n] = 1.0
    ones = np.ones((1, 128), np.float16)
    boff = b_off.reshape(27, 1).astype(np.float32)

    # x as f16 with x-direction pad pre-applied: [B, C, H, W2]
    xp16 = np.zeros((B, C, H, W2), np.float16)
    xp16[:, :, :, P:P + W] = x.astype(np.float16)

    in_maps = []
    for core in range(N_CORES):
        b, half = core // 2, core % 2
        h0 = half * HH
        # strip row l = image row h0 + l - P, zero-padded outside [0, H)
        strip = np.zeros((C, SR, W2), np.float16)
        lo, hi = h0 - P, h0 - P + SR
        slo, shi = max(lo, 0), min(hi, H)
        strip[:, slo - lo: slo - lo + (shi - slo)] = xp16[b, :, slo:shi]
        # baseC [128, 16, 32]: cols 0-8 strip-y base, 9-17 padded-x base
        basec = np.zeros((128, 16, 32), np.float32)
        pp_ = np.arange(128)
        for ch in range(16):
            s_ = ch * 128 + pp_
            rloc = s_ // W          # local row 0..31
            wloc = s_ % W
            for k in range(K):
                basec[:, ch, k] = rloc + (k // 3) - 1 + P
                basec[:, ch, 9 + k] = wloc + (k % 3) - 1 + P
        in_maps.append({
            "xstrip": strip,
            "w_t": wt,
            "woff_t": wofft,
            "b_off": boff,
            "bnvec": bnv,
            "baseC": basec.reshape(128, 16 * 32),
            "ident32": id32,
            "ident16": id16,
            "perm32": perm,
            "ones16": ones,
        })
    return in_maps


def _get_nc():
    if "nc" not in _NC_CACHE:
        _NC_CACHE["nc"] = build_nc()
    return _NC_CACHE["nc"]


def kernel(**inputs):
    nc = _get_nc()
    in_maps = _host_prep(inputs)
    res = bass_utils.run_bass_kernel_spmd(nc, in_maps, core_ids=list(range(N_CORES)))
    out = np.zeros((B, O, H, W), np.float32)
    for core in range(N_CORES):
        b, half = core // 2, core % 2
        arr = res.results[core]["out_c"].reshape(S, O)  # s = ch*128 + p
        out[b, :, half * HH:(half + 1) * HH, :] = (
            arr.reshape(HH, W, O).transpose(2, 0, 1))
    return out


# revision 20
# speedup vs baseline: 1.2917x; 1.2065x over previous
"""Trainium2 Bass kernel: modulated deformable conv 3x3 (DCNv2) + BN(eval)
+ ReLU.  B=4, C=O=256, H=W=64, distributed over 8 NeuronCores.

Sharding: core i -> batch b = i//2, image row-half = i%2 (32 rows). Each core
computes out[b, :, h0:h0+32, :] fully (data-parallel over batch x row-half).

Design v4 (from v2 @316us; bottleneck = Q7 SWDGE descriptor gen ~10ns/idx,
which is serial on the Pool engine regardless of queue_num — measured):
  - Per-core 41-row STRIP input (h0-aligned, zero-padded) instead of full
    image + separate halo: om conv reads the strip directly (xhalo input
    and its 1.2MB load die), XY2 scratch shrinks 5504->3072 tokens
    (24 transposes-chunks instead of 43, -2.5MB writes).
  - Preamble split by sample-half: om banks 0-1 -> omT/math/idx for half 0
    -> first gathers at ~30us (vs 60); om banks 2-3 + half-1 chain overlap
    the first gathers.
  - 18 gather calls (9 taps x 2 halves, 1024 idxs each) instead of 27
    smaller ones: Q7 fixed cost amortized, fewer sem waits/drains.
  - Main loop per (half, k): gather [128,8,1024] -> 4 mults + 3 adds (DVE,
    dup-pair 2x APs) -> 16 PE transposes V^T->V -> ACT copy -> 16
    accumulating matmuls into per-chunk-pair PSUM acc tiles; bias via
    K=1 matmul; ReLU on ACT; per-half output DMA.
  - XY2 DRAM scratch [3072 tokens, 512 f16]: token t=(l,x) holds
    [strip^T at (l,x) ; strip^T at (l+1,x)] (row-pair interleave, 1024B
    stride). dma_gather elem_size=2048B / elem_step=1024B fetches all 4
    bilinear corners with one descriptor per (tap, sample).
  - Indices: 16x8 bit-swap folded into PE transpose via perm matrix pm32;
    both idx DMA hops are plain 3D contiguous copies.
"""

import numpy as np

import concourse.bass as bass
import concourse.bacc as bacc
import concourse.mybir as mybir
import concourse.tile as tile
from concourse.ap import AP
from concourse import bass_utils

F32 = mybir.dt.float32
F16 = mybir.dt.float16
I16 = mybir.dt.int16
I32 = mybir.dt.int32
AF = mybir.ActivationFunctionType
ALU = mybir.AluOpType

B, C, O, H, W = 4, 256, 256, 64, 64
K = 9
P = 5                     # x-pad (cols) and strip top margin
W2 = W + 2 * P            # 74
SR = 41                   # strip rows per core (h0-aligned, padded coords)
NT = 40 * W2              # 2960 usable tokens (l in [0,40))
NQP = 3072                # 24*128 padded token count
HH = 32                   # output rows per core
S = HH * W                # 2048 samples per core
CC = C // 128             # 2
NH = 2                    # sample halves
SH = S // NH              # 1024 samples per half (8 chunks of 128)
JH = 8                    # chunks per half
EPS = 1e-5
N_CORES = 8

_NC_CACHE = {}


def build_nc():
    nc = bacc.Bacc("TRN2", target_bir_lowering=False, debug=False,
                   num_devices=N_CORES)

    xs_in = nc.dram_tensor("xstrip", [C, SR, W2], F16, kind="ExternalInput")
    w_t = nc.dram_tensor("w_t", [CC, 128, K * O], F32, kind="ExternalInput")
    woff_t = nc.dram_tensor("woff_t", [CC, 128, K * 27], F32, kind="ExternalInput")
    b_off_in = nc.dram_tensor("b_off", [27, 1], F32, kind="ExternalInput")
    bnvec = nc.dram_tensor("bnvec", [1, 5 * O], F32, kind="ExternalInput")
    baseC = nc.dram_tensor("baseC", [128, 16 * 32], F32, kind="ExternalInput")
    ident32 = nc.dram_tensor("ident32", [128, 128], F32, kind="ExternalInput")
    ident16 = nc.dram_tensor("ident16", [128, 128], F16, kind="ExternalInput")
    perm32 = nc.dram_tensor("perm32", [128, 128], F32, kind="ExternalInput")
    ones16 = nc.dram_tensor("ones16", [1, 128], F16, kind="ExternalInput")

    out_d = nc.dram_tensor("out_c", [S, O], F32, kind="ExternalOutput")

    with tile.TileContext(nc) as tc:
        _build(nc, tc, xs_in, w_t, woff_t, b_off_in, bnvec, baseC,
               ident32, ident16, perm32, ones16, out_d)
    nc.compile()
    return nc


def _build(nc, tc, xs_in, w_t, woff_t, b_off_in, bnvec, baseC,
           ident32, ident16, perm32, ones16, out_d):
    from contextlib import ExitStack

    with ExitStack() as top:
        pers = top.enter_context(tc.tile_pool(name="pers", bufs=1))
        dram = top.enter_context(tc.tile_pool(name="dram", bufs=1, space="DRAM"))
        xy2_t = dram.tile([NQP, 512], F16, name="xy2_scr", tag="xy2")
        idxs_t = dram.tile([K, 16, 128], I16, name="idx_scr", tag="idxs")
        ph1_cm = tc.tile_pool(name="ph1", bufs=1)
        ph1 = ph1_cm.__enter__()

        # ------------- om-gating loads FIRST (strip + offset weights) -------
        woff16 = []
        for cc in range(CC):
            woff16.append(ph1.tile([128, K * 27], F16, name=f"woff{cc}", tag=f"woff{cc}"))
        xst = []
        xs_rows = [0, 11, 21, 31, 41]
        for cc in range(CC):
            t = ph1.tile([128, NQP], F16, name=f"xst{cc}", tag=f"xst{cc}")
            for a, b in zip(xs_rows[:-1], xs_rows[1:]):
                nc.sync.dma_start(
                    out=t[:, a * W2:b * W2],
                    in_=xs_in.ap()[cc * 128:(cc + 1) * 128, a:b].rearrange(
                        "p h w -> p (h w)"))
            nc.vector.memset(t[:, SR * W2:NQP], 0.0)
            xst.append(t)
        with tc.tile_pool(name="wotmp", bufs=1) as wotmp:
            for cc in range(CC):
                wo = wotmp.tile([128, K * 27], F32, name=f"wo{cc}", tag="wo")
                nc.sync.dma_start(out=wo[:], in_=woff_t.ap()[cc])
                nc.vector.tensor_copy(woff16[cc][:], wo[:])

        # ------------- constants -------------
        id32 = pers.tile([128, 128], F32)
        nc.sync.dma_start(out=id32[:], in_=ident32.ap())
        id16 = pers.tile([128, 128], F16)
        nc.sync.dma_start(out=id16[:], in_=ident16.ap())
        pm32 = pers.tile([128, 128], F32)
        nc.sync.dma_start(out=pm32[:], in_=perm32.ap())
        one16 = pers.tile([1, 128], F16)
        nc.sync.dma_start(out=one16[:], in_=ones16.ap())
        base_t = pers.tile([128, 16, 32], F32)
        nc.sync.dma_start(out=base_t[:], in_=baseC.ap().rearrange("p (a b) -> p a b", a=16))
        boff_t = pers.tile([27, 1], F32)
        nc.sync.dma_start(out=boff_t[:], in_=b_off_in.ap())
        # preload the Sigmoid ACT table before it's on the critical chain
        sigdum = pers.tile([1, 1], F32)
        nc.scalar.activation(sigdum[:], boff_t[0:1, 0:1], AF.Sigmoid)
        # dummy gather: pay the one-time Q7 IRAM library load (~9us) now,
        # while the preamble runs, instead of at the first real gather.
        # Reads a private DRAM scratch so no WAR against the XY2 build.
        dum_scr = dram.tile([128, 128], F16, name="dum_scr", tag="dum")
        dumidx = pers.tile([128, 8], I16)
        nc.vector.memset(dumidx[:], 0)
        dumg = pers.tile([128, 1, 128], F16)
        nc.gpsimd.dma_gather(
            dumg[:], AP(dum_scr.tensor, 0, [(128, 127), (1, 128)]),
            dumidx[:], 128, 128, 128, single_packet=False)

        # ------------- offset conv: om [27, 2048], banks of 512 -------------
        # bank bk covers local rows 8bk..8bk+8; rhs rows (strip) 4+8bk+ty.
        om_sb = ph1.tile([27, S], F32)
        omps_cm = tc.tile_pool(name="omps", bufs=1, space="PSUM")
        omps = omps_cm.__enter__()
        om_ps = omps.tile([27, S], F32, name="om_ps", tag="om_ps")
        for bk in range(4):
            for cc in range(CC):
                for t9 in range(K):
                    ty, tx = t9 // 3, t9 % 3
                    rhs = xst[cc][:, 0:SR * W2].rearrange("p (h w) -> p h w", w=W2)[
                        :, bk * 8 + 4 + ty: bk * 8 + 4 + ty + 8,
                        P - 1 + tx: P - 1 + tx + W]
                    nc.tensor.matmul(om_ps[:, bk * 512:(bk + 1) * 512],
                                     woff16[cc][:, t9 * 27:(t9 + 1) * 27], rhs,
                                     start=(cc == 0 and t9 == 0),
                                     stop=(cc == CC - 1 and t9 == K - 1))
        # per-half copies so half-0 math starts while banks 2-3 still run
        for h in range(NH):
            nc.scalar.activation(om_sb[:, h * SH:(h + 1) * SH],
                                 om_ps[:, h * SH:(h + 1) * SH], AF.Identity,
                                 bias=boff_t[:, 0:1])

        # ------------- XY2 build: token t = [XT[t]; XT[t+74]] ---------------
        with tc.tile_pool(name="xtp", bufs=3) as xtp, \
             tc.tile_pool(name="xtps", bufs=3, space="PSUM") as xtps:
            for grp in range(6):           # 4 q-chunks per group, 24 chunks
                qcs = list(range(grp * 4, grp * 4 + 4))
                pt = xtps.tile([128, 1024], F16, name=f"xt_ps{grp}", tag="xt_ps")
                for i, qc in enumerate(qcs):
                    for cc in range(CC):
                        nc.tensor.transpose(
                            pt[:, i * 256 + cc * 128: i * 256 + (cc + 1) * 128],
                            xst[cc][:, qc * 128:(qc + 1) * 128], id16[:])
                st = xtp.tile([128, 1024], F16, name=f"xt_sb{grp}", tag="xt_sb")
                nc.scalar.activation(st[:], pt[:], AF.Copy)
                a = grp * 512
                src3 = st[:].rearrange("p (qc c) -> p qc c", c=256)
                # write 1: first half of tokens a..a+512
                dst1 = xy2_t[a: a + 512, 0:256].rearrange("(qc p) c -> p qc c", p=128)
                nc.sync.dma_start(out=dst1, in_=src3)
                # write 2: second half of tokens a-74..a+512-74
                if grp == 0:
                    dst2a = xy2_t[0:54, 256:512]
                    nc.sync.dma_start(out=dst2a, in_=st[74:128, 0:256])
                    dst2b = xy2_t[54: 54 + 384, 256:512].rearrange(
                        "(qc p) c -> p qc c", p=128)
                    nc.sync.dma_start(
                        out=dst2b,
                        in_=st[:, 256:1024].rearrange("p (qc c) -> p qc c", c=256))
                else:
                    dst2 = xy2_t[a - 74: a - 74 + 512, 256:512].rearrange(
                        "(qc p) c -> p qc c", p=128)
                    nc.sync.dma_start(out=dst2, in_=src3)

        # ------------- per-half: omT, sample math, weights, indices ---------
        # all sample-major [128, 8, .]; wqs/wrapped persist for the main loop
        wqs = pers.tile([128, 16, 9, 4, 2], F16, name="wqs", tag="wqs")
        wrapped_all = pers.tile([128, K, 128], I16, name="wrapped", tag="wrapped")
        omT_all = ph1.tile([128, 16, 32], F32, name="omT_all", tag="omT_all")

        with tc.tile_pool(name="mth", bufs=2) as mth, \
             tc.tile_pool(name="mthps", bufs=2, space="PSUM") as mthps:
            for h in range(NH):
                c0 = h * JH
                omT_ps = mthps.tile([128, 256], F32, name=f"omT{h}", tag="omT_ps")
                nc.vector.memset(omT_ps[:], 0.0)
                for ch in range(JH):
                    nc.tensor.transpose(
                        omT_ps[:, ch * 32: ch * 32 + 27],
                        om_sb[:, (c0 + ch) * 128:(c0 + ch + 1) * 128],
                        id32[0:27, 0:27])
                omT = omT_ps[:].rearrange("p (a b) -> p a b", a=JH)
                nc.vector.tensor_tensor(out=omT_all[:, c0:c0 + JH, :], in0=omT,
                                        in1=base_t[:, c0:c0 + JH, :], op=ALU.add)
                ppx = omT_all[:, c0:c0 + JH, :]
                ii = mth.tile([128, JH, 18], I32, name=f"ii{h}", tag="ii")
                nc.vector.tensor_copy(ii[:], ppx[:, :, 0:18])
                ff = mth.tile([128, JH, 18], F32, name=f"ff{h}", tag="ff")
                nc.vector.tensor_copy(ff[:], ii[:])
                gtt = mth.tile([128, JH, 18], F32, name=f"gtt{h}", tag="gtt")
                nc.vector.tensor_tensor(out=gtt[:], in0=ff[:], in1=ppx[:, :, 0:18],
                                        op=ALU.is_gt)
                flo = mth.tile([128, JH, 18], F32, name=f"flo{h}", tag="flo")
                nc.vector.tensor_tensor(out=flo[:], in0=ff[:], in1=gtt[:],
                                        op=ALU.subtract)
                lf = mth.tile([128, JH, 18], F32, name=f"lf{h}", tag="lf")
                nc.vector.tensor_tensor(out=lf[:], in0=ppx[:, :, 0:18], in1=flo[:],
                                        op=ALU.subtract)
                floc = mth.tile([128, JH, 18], F32, name=f"floc{h}", tag="floc")
                # clip y to [0, 39], x to [0, 72]: columns 0:9 are y, 9:18 x
                nc.vector.tensor_scalar(floc[:, :, 0:9], flo[:, :, 0:9],
                                        0.0, 39.0, ALU.max, ALU.min)
                nc.vector.tensor_scalar(floc[:, :, 9:18], flo[:, :, 9:18],
                                        0.0, float(W2 - 2), ALU.max, ALU.min)
                msk = mth.tile([128, JH, 9], F32, name=f"msk{h}", tag="msk")
                nc.scalar.activation(msk[:], omT[:, :, 18:27], AF.Sigmoid)
                ol = mth.tile([128, JH, 18], F32, name=f"ol{h}", tag="ol")
                nc.vector.tensor_scalar(ol[:], lf[:], -1.0, 1.0, ALU.mult, ALU.add)
                # corner weights (mask folded), slot s = xa*2 + ya, DUPLICATED
                # adjacent f16 pairs for the DVE 2x packed-mode combine APs
                for sq in range(4):
                    ya, xa = sq % 2, sq // 2
                    yw = ol if ya == 0 else lf
                    xw = ol if xa == 0 else lf
                    wtmp32 = mth.tile([128, JH, 9], F32, name=f"wrt{h}_{sq}", tag="wrt")
                    nc.vector.tensor_tensor(out=wtmp32[:], in0=yw[:, :, 0:9],
                                            in1=xw[:, :, 9:18], op=ALU.mult)
                    nc.vector.tensor_tensor(
                        out=wqs[:, c0:c0 + JH, :, sq, :],
                        in0=wtmp32[:].unsqueeze(3).broadcast_to([128, JH, 9, 2]),
                        in1=msk[:].unsqueeze(3).broadcast_to([128, JH, 9, 2]),
                        op=ALU.mult)
                # token id t = y0c*74 + x0c  [128, 8, 9]
                tokf = mth.tile([128, JH, 9], F32, name=f"tokf{h}", tag="tokf")
                nc.vector.tensor_scalar(tokf[:], floc[:, :, 0:9], float(W2), None,
                                        ALU.mult)
                nc.vector.tensor_tensor(out=tokf[:], in0=tokf[:],
                                        in1=floc[:, :, 9:18], op=ALU.add)
                # index build: PE transpose with perm matrix folds the 16x8
                # bit-swap; then two plain DMA hops to wrapped layout
                tps = mthps.tile([JH * K, 128], F32, name=f"tk{h}", tag="tkps")
                nc.tensor.transpose(
                    tps[:], tokf[:].rearrange("p a b -> p (a b)"), pm32[:])
                t16 = mth.tile([JH * K, 128], I16, name=f"t16_{h}", tag="t16")
                nc.vector.tensor_copy(t16[:], tps[:])
                # hop1 DMAs on the scalar queue (dodges Sync congestion)
                for jl in range(JH):
                    nc.scalar.dma_start(
                        out=idxs_t[:, :, (c0 + jl) * 8:(c0 + jl + 1) * 8],
                        in_=t16[jl * K:(jl + 1) * K, :].rearrange(
                            "k (w g) -> k w g", g=8))
                # hop2: queue-0 gather reads idxs via 32 channels -> 2 replicas
                for rep in range(2):
                    nc.scalar.dma_start(
                        out=wrapped_all[rep * 16:(rep + 1) * 16, :,
                                        h * 64:(h + 1) * 64],
                        in_=idxs_t[:, :, h * 64:(h + 1) * 64].rearrange(
                            "k w f -> w k f"))

        omps_cm.__exit__(None, None, None)

        # ------------- BN fold -------------
        bn_t = pers.tile([1, 5 * O], F32)
        nc.sync.dma_start(out=bn_t[:], in_=bnvec.ap())
        gam = bn_t[:, 0:O]; bet = bn_t[:, O:2 * O]; rmn = bn_t[:, 2 * O:3 * O]
        rvr = bn_t[:, 3 * O:4 * O]; bia = bn_t[:, 4 * O:5 * O]
        sq = pers.tile([1, O], F32)
        nc.vector.tensor_scalar(sq[:], rvr, float(EPS), None, ALU.add)
        nc.scalar.activation(sq[:], sq[:], AF.Sqrt)
        sfac = pers.tile([1, O], F32)
        nc.vector.reciprocal(sfac[:], sq[:])
        nc.vector.tensor_tensor(out=sfac[:], in0=sfac[:], in1=gam, op=ALU.mult)
        bpr = pers.tile([1, O], F32)
        nc.vector.tensor_tensor(out=bpr[:], in0=bia, in1=rmn, op=ALU.subtract)
        nc.vector.tensor_tensor(out=bpr[:], in0=bpr[:], in1=sfac[:], op=ALU.mult)
        nc.vector.tensor_tensor(out=bpr[:], in0=bpr[:], in1=bet, op=ALU.add)
        bprow16 = pers.tile([1, O], F16)
        nc.vector.tensor_copy(bprow16[:], bpr[:])
        # bias duplicated [1, 2*O] so one start=True matmul initializes a
        # whole chunk-pair PSUM bank (bank-wide clear happens exactly once)
        bprow16d = pers.tile([1, 2 * O], F16)
        nc.vector.tensor_copy(bprow16d[:, 0:O], bpr[:])
        nc.vector.tensor_copy(bprow16d[:, O:2 * O], bpr[:])
        sbc = pers.tile([128, O], F32)
        ones32 = pers.tile([1, 128], F32)
        nc.vector.memset(ones32[:], 1.0)
        with tc.tile_pool(name="bcp", bufs=1, space="PSUM") as bcp:
            bc_ps = bcp.tile([128, O], F32)
            nc.tensor.matmul(bc_ps[:], ones32[:], sfac[:], start=True, stop=True)
            nc.vector.tensor_copy(sbc[:], bc_ps[:])

        # ------------- main weights (BN-scaled, f16) -------------
        wmain = []
        for cc in range(CC):
            wmain.append(pers.tile([128, K * O], F16, name=f"wmain{cc}", tag=f"wmain{cc}"))
        with tc.tile_pool(name="wtmp", bufs=1) as wtmp:
            for cc in range(CC):
                wr = wtmp.tile([128, K * O], F32, name=f"wr{cc}", tag="wr")
                nc.sync.dma_start(out=wr[:], in_=w_t.ap()[cc])
                for k in range(K):
                    nc.vector.tensor_tensor(out=wmain[cc][:, k * O:(k + 1) * O],
                                            in0=wr[:, k * O:(k + 1) * O],
                                            in1=sbc[:], op=ALU.mult)

        # ------------- main loop: per (half, tap) -------------
        ph1_cm.__exit__(None, None, None)
        gsrc = AP(xy2_t.tensor, 0, [(512, NQP - 1), (1, 1024)])

        with tc.tile_pool(name="mg", bufs=3) as mg, \
             tc.tile_pool(name="mv", bufs=2) as mv, \
             tc.tile_pool(name="mvs", bufs=3) as mvs, \
             tc.tile_pool(name="mo", bufs=2) as mo, \
             tc.tile_pool(name="mps", bufs=1, space="PSUM") as mps, \
             tc.tile_pool(name="accp", bufs=1, space="PSUM") as accp:
            for h in range(NH):
                c0 = h * JH
                accs = [accp.tile([128, 512], F32, name=f"acc{h}_{pj}",
                                  tag=f"acc{pj}") for pj in range(4)]
                # bias-init each chunk-pair bank (start=True exactly once per
                # bank; all tap matmuls then accumulate with start=False)
                for pj in range(4):
                    nc.tensor.matmul(accs[pj][:], one16[:], bprow16d[:],
                                     start=True, stop=False)
                outT = mo.tile([128, JH, O], F32, name=f"outT{h}", tag="outT")
                for k in range(K):
                    gt = mg.tile([128, JH, 1024], F16, name=f"g{h}_{k}", tag="gt")
                    nc.gpsimd.dma_gather(
                        gt[:], gsrc,
                        wrapped_all[:, k, h * 64:(h + 1) * 64],
                        SH, SH, 1024,
                        elem_step=512, single_packet=False)
                    # combine: 4 per-slot mults (2x-eligible dup-pair APs)
                    # + a 3-add tree
                    m4 = mv.tile([128, JH, 4, 256], F16, name=f"m4_{h}_{k}",
                                 tag="m4")
                    vt = mv.tile([128, JH, 256], F16, name=f"v{h}_{k}", tag="vt")
                    for sl in range(4):
                        g_sl = gt[:, :, sl * 256:(sl + 1) * 256].rearrange(
                            "p j (r d) -> p j r d", d=2)
                        w_sl = (wqs[:, c0:c0 + JH, k, sl, :]
                                .unsqueeze(2).broadcast_to([128, JH, 128, 2]))
                        o_sl = m4[:, :, sl, :].rearrange(
                            "p j (r d) -> p j r d", d=2)
                        nc.vector.tensor_tensor(out=o_sl, in0=g_sl,
                                                in1=w_sl, op=ALU.mult)
                    # pairwise tree in-place: one [.,2,256] add + one final
                    # add (6 DVE ops per tap instead of 7, no extra tile)
                    nc.vector.tensor_tensor(out=m4[:, :, 0:2, :],
                                            in0=m4[:, :, 0:2, :],
                                            in1=m4[:, :, 2:4, :], op=ALU.add)
                    nc.vector.tensor_tensor(out=vt[:], in0=m4[:, :, 0, :],
                                            in1=m4[:, :, 1, :], op=ALU.add)
                    # transpose V^T[s,c] -> V[c,s] (JH*2 x [128,128])
                    vps = mps.tile([128, JH * 2 * 128], F16, name=f"vps{h}_{k}",
                                   tag="vps")
                    for j in range(JH):
                        for cc in range(CC):
                            nc.tensor.transpose(
                                vps[:, (cc * JH + j) * 128:(cc * JH + j + 1) * 128],
                                vt[:, j, cc * 128:(cc + 1) * 128], id16[:])
                    vsb = mvs.tile([128, JH * 2 * 128], F16, name=f"vs{h}_{k}",
                                   tag="vsb")
                    nc.scalar.activation(vsb[:], vps[:], AF.Copy)
                    # accumulate this tap into the 8 per-chunk acc slices;
                    # on the last tap, stream each bank out as it completes
                    for j in range(JH):
                        acc = accs[j // 2][:, (j % 2) * 256:(j % 2) * 256 + 256]
                        for cc in range(CC):
                            nc.tensor.matmul(
                                acc,
                                vsb[:, (cc * JH + j) * 128:(cc * JH + j + 1) * 128],
                                wmain[cc][:, k * O:(k + 1) * O],
                                start=False,
                                stop=(k == K - 1 and cc == CC - 1 and j % 2 == 1))
                        if k == K - 1 and j % 2 == 1:
                            pj = j // 2
                            nc.scalar.activation(
                                outT[:].rearrange("p a b -> p (a b)")[
                                    :, pj * 512:(pj + 1) * 512],
                                accs[pj][:], AF.Relu)
                            nc.sync.dma_start(
                                out=out_d.ap()[
                                    (c0 + 2 * pj) * 128:(c0 + 2 * pj + 2) * 128,
                                    :].rearrange("(ch p) o -> p ch o", p=128),
                                in_=outT[:, 2 * pj:2 * pj + 2, :])


# ===================== host side =====================

def _host_prep(inputs):
    """Build the 8 per-core input maps (layout-only host work + constants)."""
    x = np.ascontiguousarray(inputs["x"], dtype=np.float32)
    w_off = np.asarray(inputs["w_off"], np.float32)
    b_off = np.asarray(inputs["b_off"], np.float32)
    weight = np.asarray(inputs["weight"], np.float32)
    bias = np.asarray(inputs["bias"], np.float32)
    gamma = np.asarray(inputs["gamma"], np.float32)
    beta = np.asarray(inputs["beta"], np.float32)
    run_mean = np.asarray(inputs["run_mean"], np.float32)
    run_var = np.asarray(inputs["run_var"], np.float32)

    # weight [O, C, 3, 3] -> [CC, 128c, K, O] -> [CC, 128, K*O]
    wt = weight.reshape(O, C, K).transpose(1, 2, 0).reshape(CC, 128, K * O)
    wt = np.ascontiguousarray(wt)
    wofft = w_off.reshape(27, C, K).transpose(1, 2, 0).reshape(CC, 128, K * 27)
    wofft = np.ascontiguousarray(wofft)
    bnv = np.concatenate([gamma, beta, run_mean, run_var, bias]).astype(np.float32).reshape(1, 5 * O)
    id32 = np.eye(128, dtype=np.float32)
    id16 = np.eye(128, dtype=np.float16)
    perm = np.zeros((128, 128), np.float32)
    for n in range(128):
        perm[(n % 8) * 16 + n // 8, n] = 1.0
    ones = np.ones((1, 128), np.float16)
    boff = b_off.reshape(27, 1).astype(np.float32)

    # x as f16 with x-direction pad pre-applied: [B, C, H, W2]
    xp16 = np.zeros((B, C, H, W2), np.float16)
    xp16[:, :, :, P:P + W] = x.astype(np.float16)

    in_maps = []
    for core in range(N_CORES):
        b, half = core // 2, core % 2
        h0 = half * HH
        # strip row l = image row h0 + l - P, zero-padded outside [0, H)
        strip = np.zeros((C, SR, W2), np.float16)
        lo, hi = h0 - P, h0 - P + SR
        slo, shi = max(lo, 0), min(hi, H)
        strip[:, slo - lo: slo - lo + (shi - slo)] = xp16[b, :, slo:shi]
        # baseC [128, 16, 32]: cols 0-8 strip-y base, 9-17 padded-x base
        basec = np.zeros((128, 16, 32), np.float32)
        pp_ = np.arange(128)
        for ch in range(16):
            s_ = ch * 128 + pp_
            rloc = s_ // W          # local row 0..31
            wloc = s_ % W
            for k in range(K):
                basec[:, ch, k] = rloc + (k // 3) - 1 + P
                basec[:, ch, 9 + k] = wloc + (k % 3) - 1 + P
        in_maps.append({
            "xstrip": strip,
            "w_t": wt,
            "woff_t": wofft,
            "b_off": boff,
            "bnvec": bnv,
            "baseC": basec.reshape(128, 16 * 32),
            "ident32": id32,
            "ident16": id16,
            "perm32": perm,
            "ones16": ones,
        })
    return in_maps


def _get_nc():
    if "nc" not in _NC_CACHE:
        _NC_CACHE["nc"] = build_nc()
    return _NC_CACHE["nc"]


def kernel(**inputs):
    nc = _get_nc()
    in_maps = _host_prep(inputs)
    res = bass_utils.run_bass_kernel_spmd(nc, in_maps, core_ids=list(range(N_CORES)))
    out = np.zeros((B, O, H, W), np.float32)
    for core in range(N_CORES):
        b, half = core // 2, core % 2
        arr = res.results[core]["out_c"].reshape(S, O)  # s = ch*128 + p
        out[b, :, half * HH:(half + 1) * HH, :] = (
            arr.reshape(HH, W, O).transpose(2, 0, 1))
    return out


# revision 34
# speedup vs baseline: 1.4270x; 1.1048x over previous
"""Trainium2 Bass kernel: modulated deformable conv 3x3 (DCNv2) + BN(eval)
+ ReLU.  B=4, C=O=256, H=W=64, distributed over 8 NeuronCores.

Sharding: core i -> batch b = i//2, image row-half = i%2 (32 rows). Each core
computes out[b, :, h0:h0+32, :] fully (data-parallel over batch x row-half).

Design v4 (from v2 @316us; bottleneck = Q7 SWDGE descriptor gen ~10ns/idx,
which is serial on the Pool engine regardless of queue_num — measured):
  - Per-core 41-row STRIP input (h0-aligned, zero-padded) instead of full
    image + separate halo: om conv reads the strip directly (xhalo input
    and its 1.2MB load die), XY2 scratch shrinks 5504->3072 tokens
    (24 transposes-chunks instead of 43, -2.5MB writes).
  - Preamble split by sample-half: om banks 0-1 -> omT/math/idx for half 0
    -> first gathers at ~30us (vs 60); om banks 2-3 + half-1 chain overlap
    the first gathers.
  - 18 gather calls (9 taps x 2 halves, 1024 idxs each) instead of 27
    smaller ones: Q7 fixed cost amortized, fewer sem waits/drains.
  - Main loop per (half, k): gather [128,8,1024] -> 4 mults + 3 adds (DVE,
    dup-pair 2x APs) -> 16 PE transposes V^T->V -> ACT copy -> 16
    accumulating matmuls into per-chunk-pair PSUM acc tiles; bias via
    K=1 matmul; ReLU on ACT; per-half output DMA.
  - XY2 DRAM scratch [3072 tokens, 512 f16]: token t=(l,x) holds
    [strip^T at (l,x) ; strip^T at (l+1,x)] (row-pair interleave, 1024B
    stride). dma_gather elem_size=2048B / elem_step=1024B fetches all 4
    bilinear corners with one descriptor per (tap, sample).
  - Indices: 16x8 bit-swap folded into PE transpose via perm matrix pm32;
    both idx DMA hops are plain 3D contiguous copies.
"""

import numpy as np

import concourse.bass as bass
import concourse.bacc as bacc
import concourse.mybir as mybir
import concourse.tile as tile
from concourse.ap import AP
from concourse import bass_utils

F32 = mybir.dt.float32
F16 = mybir.dt.float16
I16 = mybir.dt.int16
I32 = mybir.dt.int32
AF = mybir.ActivationFunctionType
ALU = mybir.AluOpType

B, C, O, H, W = 4, 256, 256, 64, 64
K = 9
P = 5                     # x-pad (cols) and strip top margin
W2 = W + 2 * P            # 74
SR = 41                   # strip rows per core (h0-aligned, padded coords)
NT = 40 * W2              # 2960 usable tokens (l in [0,40))
NQP = 3072                # 24*128 padded token count
HH = 32                   # output rows per core
S = HH * W                # 2048 samples per core
CC = C // 128             # 2
NH = 2                    # sample halves
SH = S // NH              # 1024 samples per half (8 chunks of 128)
JH = 8                    # chunks per half
EPS = 1e-5
N_CORES = 8

_NC_CACHE = {}


def build_nc():
    nc = bacc.Bacc("TRN2", target_bir_lowering=False, debug=False,
                   num_devices=N_CORES)

    xs_in = nc.dram_tensor("xstrip", [C, SR, W2], F16, kind="ExternalInput")
    w_t = nc.dram_tensor("w_t", [CC, 128, K * O], F16, kind="ExternalInput")
    woff_t = nc.dram_tensor("woff_t", [CC, 128, K * 27], F32, kind="ExternalInput")
    b_off_in = nc.dram_tensor("b_off", [27, 1], F32, kind="ExternalInput")
    bprow_in = nc.dram_tensor("bprow", [1, 2 * O], F16, kind="ExternalInput")
    baseC = nc.dram_tensor("baseC", [128, 16 * 32], F32, kind="ExternalInput")
    ident32 = nc.dram_tensor("ident32", [128, 128], F32, kind="ExternalInput")
    ident16 = nc.dram_tensor("ident16", [128, 128], F16, kind="ExternalInput")
    perm32 = nc.dram_tensor("perm32", [128, 128], F32, kind="ExternalInput")
    ones16 = nc.dram_tensor("ones16", [1, 128], F16, kind="ExternalInput")

    out_d = nc.dram_tensor("out_c", [S, O], F32, kind="ExternalOutput")

    with tile.TileContext(nc) as tc:
        _build(nc, tc, xs_in, w_t, woff_t, b_off_in, bprow_in, baseC,
               ident32, ident16, perm32, ones16, out_d)
    nc.compile()
    return nc


def _build(nc, tc, xs_in, w_t, woff_t, b_off_in, bprow_in, baseC,
           ident32, ident16, perm32, ones16, out_d):
    from contextlib import ExitStack

    with ExitStack() as top:
        pers = top.enter_context(tc.tile_pool(name="pers", bufs=1))
        dram = top.enter_context(tc.tile_pool(name="dram", bufs=1, space="DRAM"))
        xy2_t = dram.tile([NQP, 512], F16, name="xy2_scr", tag="xy2")
        # idx staging: plain copy of t16 per half (hop1 is one 2D DMA; hop2
        # un-scrambles with an unconstrained 4D DRAM-source AP)
        idxs_t = dram.tile([NH, JH * K, 128], I16, name="idx_scr", tag="idxs")
        ph1_cm = tc.tile_pool(name="ph1", bufs=1)
        ph1 = ph1_cm.__enter__()

        # ------------- om-gating loads FIRST (strip + offset weights) -------
        woff16 = []
        for cc in range(CC):
            woff16.append(ph1.tile([128, K * 27], F16, name=f"woff{cc}", tag=f"woff{cc}"))
        xst = []
        xs_rows = [0, 11, 21, 31, 41]
        for cc in range(CC):
            t = ph1.tile([128, NQP], F16, name=f"xst{cc}", tag=f"xst{cc}")
            for a, b in zip(xs_rows[:-1], xs_rows[1:]):
                nc.sync.dma_start(
                    out=t[:, a * W2:b * W2],
                    in_=xs_in.ap()[cc * 128:(cc + 1) * 128, a:b].rearrange(
                        "p h w -> p (h w)"))
            nc.vector.memset(t[:, SR * W2:NQP], 0.0)
            xst.append(t)
        with tc.tile_pool(name="wotmp", bufs=1) as wotmp:
            for cc in range(CC):
                wo = wotmp.tile([128, K * 27], F32, name=f"wo{cc}", tag="wo")
                nc.sync.dma_start(out=wo[:], in_=woff_t.ap()[cc])
                nc.vector.tensor_copy(woff16[cc][:], wo[:])

        # ------------- constants -------------
        id32 = pers.tile([128, 128], F32)
        nc.sync.dma_start(out=id32[:], in_=ident32.ap())
        id16 = pers.tile([128, 128], F16)
        nc.sync.dma_start(out=id16[:], in_=ident16.ap())
        pm32 = pers.tile([128, 128], F32)
        nc.sync.dma_start(out=pm32[:], in_=perm32.ap())
        one16 = pers.tile([1, 128], F16)
        nc.sync.dma_start(out=one16[:], in_=ones16.ap())
        base_t = pers.tile([128, 16, 32], F32)
        nc.sync.dma_start(out=base_t[:], in_=baseC.ap().rearrange("p (a b) -> p a b", a=16))
        boff_t = pers.tile([27, 1], F32)
        nc.sync.dma_start(out=boff_t[:], in_=b_off_in.ap())
        # preload the Sigmoid ACT table before it's on the critical chain
        sigdum = pers.tile([1, 1], F32)
        nc.scalar.activation(sigdum[:], boff_t[0:1, 0:1], AF.Sigmoid)
        # dummy gather: pay the one-time Q7 IRAM library load (~9us) now,
        # while the preamble runs, instead of at the first real gather.
        # Reads a private DRAM scratch so no WAR against the XY2 build.
        dum_scr = dram.tile([128, 128], F16, name="dum_scr", tag="dum")
        dumidx = pers.tile([128, 8], I16)
        nc.vector.memset(dumidx[:], 0)
        dumg = pers.tile([128, 1, 128], F16)
        nc.gpsimd.dma_gather(
            dumg[:], AP(dum_scr.tensor, 0, [(128, 127), (1, 128)]),
            dumidx[:], 128, 128, 128, single_packet=False)

        # ------------- offset conv: om [27, 2048], banks of 512 -------------
        # bank bk covers local rows 8bk..8bk+8; rhs rows (strip) 4+8bk+ty.
        om_sb = ph1.tile([27, S], F32)
        omps_cm = tc.tile_pool(name="omps", bufs=1, space="PSUM")
        omps = omps_cm.__enter__()
        om_ps = omps.tile([27, S], F32, name="om_ps", tag="om_ps")
        for bk in range(4):
            for cc in range(CC):
                for t9 in range(K):
                    ty, tx = t9 // 3, t9 % 3
                    rhs = xst[cc][:, 0:SR * W2].rearrange("p (h w) -> p h w", w=W2)[
                        :, bk * 8 + 4 + ty: bk * 8 + 4 + ty + 8,
                        P - 1 + tx: P - 1 + tx + W]
                    nc.tensor.matmul(om_ps[:, bk * 512:(bk + 1) * 512],
                                     woff16[cc][:, t9 * 27:(t9 + 1) * 27], rhs,
                                     start=(cc == 0 and t9 == 0),
                                     stop=(cc == CC - 1 and t9 == K - 1))
        # per-half copies so half-0 math starts while banks 2-3 still run
        for h in range(NH):
            nc.scalar.activation(om_sb[:, h * SH:(h + 1) * SH],
                                 om_ps[:, h * SH:(h + 1) * SH], AF.Identity,
                                 bias=boff_t[:, 0:1])

        # ------------- XY2 build: token t = [XT[t]; XT[t+74]] ---------------
        with tc.tile_pool(name="xtp", bufs=3) as xtp, \
             tc.tile_pool(name="xtps", bufs=3, space="PSUM") as xtps:
            for grp in range(6):           # 4 q-chunks per group, 24 chunks
                qcs = list(range(grp * 4, grp * 4 + 4))
                pt = xtps.tile([128, 1024], F16, name=f"xt_ps{grp}", tag="xt_ps")
                for i, qc in enumerate(qcs):
                    for cc in range(CC):
                        nc.tensor.transpose(
                            pt[:, i * 256 + cc * 128: i * 256 + (cc + 1) * 128],
                            xst[cc][:, qc * 128:(qc + 1) * 128], id16[:])
                st = xtp.tile([128, 1024], F16, name=f"xt_sb{grp}", tag="xt_sb")
                nc.scalar.activation(st[:], pt[:], AF.Copy)
                a = grp * 512
                src3 = st[:].rearrange("p (qc c) -> p qc c", c=256)
                # write 1: first half of tokens a..a+512
                dst1 = xy2_t[a: a + 512, 0:256].rearrange("(qc p) c -> p qc c", p=128)
                nc.scalar.dma_start(out=dst1, in_=src3)
                # write 2: second half of tokens a-74..a+512-74
                if grp == 0:
                    dst2a = xy2_t[0:54, 256:512]
                    nc.scalar.dma_start(out=dst2a, in_=st[74:128, 0:256])
                    dst2b = xy2_t[54: 54 + 384, 256:512].rearrange(
                        "(qc p) c -> p qc c", p=128)
                    nc.scalar.dma_start(
                        out=dst2b,
                        in_=st[:, 256:1024].rearrange("p (qc c) -> p qc c", c=256))
                else:
                    dst2 = xy2_t[a - 74: a - 74 + 512, 256:512].rearrange(
                        "(qc p) c -> p qc c", p=128)
                    nc.scalar.dma_start(out=dst2, in_=src3)

        # ------------- per-half: omT, sample math, weights, indices ---------
        # all sample-major [128, 8, .]; wqs/wrapped persist for the main loop
        wqs = pers.tile([128, 16, 9, 4, 2], F16, name="wqs", tag="wqs")
        wrapped_h = [pers.tile([128, K, 64], I16, name=f"wrapped{h}",
                               tag=f"wrapped{h}") for h in range(NH)]
        omT_all = ph1.tile([128, 16, 32], F32, name="omT_all", tag="omT_all")

        with tc.tile_pool(name="mth", bufs=2) as mth, \
             tc.tile_pool(name="mthps", bufs=2, space="PSUM") as mthps:
            for h in range(NH):
                c0 = h * JH
                omT_ps = mthps.tile([128, 256], F32, name=f"omT{h}", tag="omT_ps")
                nc.vector.memset(omT_ps[:], 0.0)
                for ch in range(JH):
                    nc.tensor.transpose(
                        omT_ps[:, ch * 32: ch * 32 + 27],
                        om_sb[:, (c0 + ch) * 128:(c0 + ch + 1) * 128],
                        id32[0:27, 0:27])
                omT = omT_ps[:].rearrange("p (a b) -> p a b", a=JH)
                nc.vector.tensor_tensor(out=omT_all[:, c0:c0 + JH, :], in0=omT,
                                        in1=base_t[:, c0:c0 + JH, :], op=ALU.add)
                ppx = omT_all[:, c0:c0 + JH, :]
                ii = mth.tile([128, JH, 18], I32, name=f"ii{h}", tag="ii")
                nc.vector.tensor_copy(ii[:], ppx[:, :, 0:18])
                ff = mth.tile([128, JH, 18], F32, name=f"ff{h}", tag="ff")
                nc.vector.tensor_copy(ff[:], ii[:])
                gtt = mth.tile([128, JH, 18], F32, name=f"gtt{h}", tag="gtt")
                nc.vector.tensor_tensor(out=gtt[:], in0=ff[:], in1=ppx[:, :, 0:18],
                                        op=ALU.is_gt)
                flo = mth.tile([128, JH, 18], F32, name=f"flo{h}", tag="flo")
                nc.vector.tensor_tensor(out=flo[:], in0=ff[:], in1=gtt[:],
                                        op=ALU.subtract)
                lf = mth.tile([128, JH, 18], F32, name=f"lf{h}", tag="lf")
                nc.vector.tensor_tensor(out=lf[:], in0=ppx[:, :, 0:18], in1=flo[:],
                                        op=ALU.subtract)
                floc = mth.tile([128, JH, 18], F32, name=f"floc{h}", tag="floc")
                # clip y to [0, 39], x to [0, 72]: columns 0:9 are y, 9:18 x
                nc.vector.tensor_scalar(floc[:, :, 0:9], flo[:, :, 0:9],
                                        0.0, 39.0, ALU.max, ALU.min)
                nc.vector.tensor_scalar(floc[:, :, 9:18], flo[:, :, 9:18],
                                        0.0, float(W2 - 2), ALU.max, ALU.min)
                msk = mth.tile([128, JH, 9], F32, name=f"msk{h}", tag="msk")
                nc.scalar.activation(msk[:], omT[:, :, 18:27], AF.Sigmoid)
                ol = mth.tile([128, JH, 18], F32, name=f"ol{h}", tag="ol")
                nc.vector.tensor_scalar(ol[:], lf[:], -1.0, 1.0, ALU.mult, ALU.add)
                # corner weights (mask folded), slot s = xa*2 + ya, DUPLICATED
                # adjacent f16 pairs for the DVE 2x packed-mode combine APs
                for sq in range(4):
                    ya, xa = sq % 2, sq // 2
                    yw = ol if ya == 0 else lf
                    xw = ol if xa == 0 else lf
                    wtmp32 = mth.tile([128, JH, 9], F32, name=f"wrt{h}_{sq}", tag="wrt")
                    nc.vector.tensor_tensor(out=wtmp32[:], in0=yw[:, :, 0:9],
                                            in1=xw[:, :, 9:18], op=ALU.mult)
                    nc.vector.tensor_tensor(
                        out=wqs[:, c0:c0 + JH, :, sq, :],
                        in0=wtmp32[:].unsqueeze(3).broadcast_to([128, JH, 9, 2]),
                        in1=msk[:].unsqueeze(3).broadcast_to([128, JH, 9, 2]),
                        op=ALU.mult)
                # token id t = y0c*74 + x0c, stored k-major [128, 9, 8] so
                # the idx hops balance to 3-dim DMA APs
                tokf = mth.tile([128, K, JH], F32, name=f"tokf{h}", tag="tokf")
                tokv = tokf[:].rearrange("p b a -> p a b")
                nc.vector.tensor_scalar(tokv, floc[:, :, 0:9], float(W2), None,
                                        ALU.mult)
                nc.vector.tensor_tensor(out=tokv, in0=tokv,
                                        in1=floc[:, :, 9:18], op=ALU.add)
                # index build: PE transpose with perm matrix folds the 16x8
                # bit-swap; then two plain DMA hops to wrapped layout
                tps = mthps.tile([JH * K, 128], F32, name=f"tk{h}", tag="tkps")
                nc.tensor.transpose(
                    tps[:], tokf[:].rearrange("p b a -> p (b a)"), pm32[:])
                t16 = mth.tile([JH * K, 128], I16, name=f"t16_{h}", tag="t16")
                nc.vector.tensor_copy(t16[:], tps[:])
                # hop1: one plain [72,128] copy to DRAM (sync queue)
                nc.sync.dma_start(out=idxs_t[h], in_=t16[:])
                # hop2: un-scramble via 4D DRAM src AP; 2 replicas for the
                # queue-0 gather's 32-channel idx read
                for rep in range(2):
                    nc.sync.dma_start(
                        out=wrapped_h[h][rep * 16:(rep + 1) * 16].rearrange(
                            "w k (j g) -> w (k j) g", g=8),
                        in_=idxs_t[h].rearrange(
                            "(k j) (w g) -> w (k j) g", k=K, g=8))

        omps_cm.__exit__(None, None, None)

        # ------------- main weights + bias (BN pre-folded on host) ----------
        bprow16d = pers.tile([1, 2 * O], F16)
        nc.sync.dma_start(out=bprow16d[:], in_=bprow_in.ap())
        wmain = []
        for cc in range(CC):
            wm = pers.tile([128, K * O], F16, name=f"wmain{cc}", tag=f"wmain{cc}")
            nc.sync.dma_start(out=wm[:], in_=w_t.ap()[cc])
            wmain.append(wm)

        # ------------- main loop: per (half, tap) -------------
        ph1_cm.__exit__(None, None, None)
        gsrc = AP(xy2_t.tensor, 0, [(512, NQP - 1), (1, 1024)])

        with tc.tile_pool(name="mg", bufs=3) as mg, \
             tc.tile_pool(name="mv", bufs=2) as mv, \
             tc.tile_pool(name="mvs", bufs=3) as mvs, \
             tc.tile_pool(name="mo", bufs=2) as mo, \
             tc.tile_pool(name="mps", bufs=1, space="PSUM") as mps, \
             tc.tile_pool(name="accp", bufs=1, space="PSUM") as accp:
            for h in range(NH):
                c0 = h * JH
                accs = [accp.tile([128, 512], F32, name=f"acc{h}_{pj}",
                                  tag=f"acc{pj}") for pj in range(4)]
                # bias-init each chunk-pair bank (start=True exactly once per
                # bank; all tap matmuls then accumulate with start=False)
                for pj in range(4):
                    nc.tensor.matmul(accs[pj][:], one16[:], bprow16d[:],
                                     start=True, stop=False)
                outT = mo.tile([128, JH, O], F32, name=f"outT{h}", tag="outT")
                for k in range(K):
                    gt = mg.tile([128, JH, 1024], F16, name=f"g{h}_{k}", tag="gt")
                    nc.gpsimd.dma_gather(
                        gt[:], gsrc,
                        wrapped_h[h][:, k, :],
                        SH, SH, 1024,
                        elem_step=512, single_packet=False)
                    # combine: 4 per-slot mults (2x-eligible dup-pair APs)
                    # + a 3-add tree
                    m4 = mv.tile([128, JH, 4, 256], F16, name=f"m4_{h}_{k}",
                                 tag="m4")
                    vt = mv.tile([128, JH, 256], F16, name=f"v{h}_{k}", tag="vt")
                    for sl in range(4):
                        g_sl = gt[:, :, sl * 256:(sl + 1) * 256].rearrange(
                            "p j (r d) -> p j r d", d=2)
                        w_sl = (wqs[:, c0:c0 + JH, k, sl, :]
                                .unsqueeze(2).broadcast_to([128, JH, 128, 2]))
                        o_sl = m4[:, :, sl, :].rearrange(
                            "p j (r d) -> p j r d", d=2)
                        nc.vector.tensor_tensor(out=o_sl, in0=g_sl,
                                                in1=w_sl, op=ALU.mult)
                    # pairwise tree in-place: one [.,2,256] add + one final
                    # add (6 DVE ops per tap instead of 7, no extra tile)
                    nc.vector.tensor_tensor(out=m4[:, :, 0:2, :],
                                            in0=m4[:, :, 0:2, :],
                                            in1=m4[:, :, 2:4, :], op=ALU.add)
                    nc.vector.tensor_tensor(out=vt[:], in0=m4[:, :, 0, :],
                                            in1=m4[:, :, 1, :], op=ALU.add)
                    # transpose V^T[s,c] -> V[c,s] (JH*2 x [128,128])
                    vps = mps.tile([128, JH * 2 * 128], F16, name=f"vps{h}_{k}",
                                   tag="vps")
                    for j in range(JH):
                        for cc in range(CC):
                            nc.tensor.transpose(
                                vps[:, (cc * JH + j) * 128:(cc * JH + j + 1) * 128],
                                vt[:, j, cc * 128:(cc + 1) * 128], id16[:])
                    vsb = mvs.tile([128, JH * 2 * 128], F16, name=f"vs{h}_{k}",
                                   tag="vsb")
                    nc.scalar.activation(vsb[:], vps[:], AF.Copy)
                    # accumulate this tap into the 8 per-chunk acc slices;
                    # on the last tap, stream each bank out as it completes
                    for j in range(JH):
                        acc = accs[j // 2][:, (j % 2) * 256:(j % 2) * 256 + 256]
                        for cc in range(CC):
                            nc.tensor.matmul(
                                acc,
                                vsb[:, (cc * JH + j) * 128:(cc * JH + j + 1) * 128],
                                wmain[cc][:, k * O:(k + 1) * O],
                                start=False,
                                stop=(k == K - 1 and cc == CC - 1 and j % 2 == 1))
                        if k == K - 1 and j % 2 == 1:
                            pj = j // 2
                            nc.scalar.activation(
                                outT[:].rearrange("p a b -> p (a b)")[
                                    :, pj * 512:(pj + 1) * 512],
                                accs[pj][:], AF.Relu)
                            nc.sync.dma_start(
                                out=out_d.ap()[
                                    (c0 + 2 * pj) * 128:(c0 + 2 * pj + 2) * 128,
                                    :].rearrange("(ch p) o -> p ch o", p=128),
                                in_=outT[:, 2 * pj:2 * pj + 2, :])


# ===================== host side =====================

def _host_prep(inputs):
    """Build the 8 per-core input maps (layout-only host work + constants)."""
    x = np.ascontiguousarray(inputs["x"], dtype=np.float32)
    w_off = np.asarray(inputs["w_off"], np.float32)
    b_off = np.asarray(inputs["b_off"], np.float32)
    weight = np.asarray(inputs["weight"], np.float32)
    bias = np.asarray(inputs["bias"], np.float32)
    gamma = np.asarray(inputs["gamma"], np.float32)
    beta = np.asarray(inputs["beta"], np.float32)
    run_mean = np.asarray(inputs["run_mean"], np.float32)
    run_var = np.asarray(inputs["run_var"], np.float32)

    # BN(eval) folded into conv weights/bias on host: y = sfac*(conv+bias-mean)+beta
    sfac = gamma / np.sqrt(run_var + EPS)
    wsc = weight.reshape(O, C * K) * sfac[:, None]
    bpr = (bias - run_mean) * sfac + beta
    # weight [O, C, 3, 3] -> [CC, 128c, K, O] -> [CC, 128, K*O], f16
    wt = wsc.reshape(O, C, K).transpose(1, 2, 0).reshape(CC, 128, K * O)
    wt = np.ascontiguousarray(wt).astype(np.float16)
    bprow = np.concatenate([bpr, bpr]).astype(np.float16).reshape(1, 2 * O)
    wofft = w_off.reshape(27, C, K).transpose(1, 2, 0).reshape(CC, 128, K * 27)
    wofft = np.ascontiguousarray(wofft)
    id32 = np.eye(128, dtype=np.float32)
    id16 = np.eye(128, dtype=np.float16)
    perm = np.zeros((128, 128), np.float32)
    for n in range(128):
        perm[(n % 8) * 16 + n // 8, n] = 1.0
    ones = np.ones((1, 128), np.float16)
    boff = b_off.reshape(27, 1).astype(np.float32)

    # x as f16 with x-direction pad pre-applied: [B, C, H, W2]
    xp16 = np.zeros((B, C, H, W2), np.float16)
    xp16[:, :, :, P:P + W] = x.astype(np.float16)

    in_maps = []
    for core in range(N_CORES):
        b, half = core // 2, core % 2
        h0 = half * HH
        # strip row l = image row h0 + l - P, zero-padded outside [0, H)
        strip = np.zeros((C, SR, W2), np.float16)
        lo, hi = h0 - P, h0 - P + SR
        slo, shi = max(lo, 0), min(hi, H)
        strip[:, slo - lo: slo - lo + (shi - slo)] = xp16[b, :, slo:shi]
        # baseC [128, 16, 32]: cols 0-8 strip-y base, 9-17 padded-x base
        basec = np.zeros((128, 16, 32), np.float32)
        pp_ = np.arange(128)
        for ch in range(16):
            s_ = ch * 128 + pp_
            rloc = s_ // W          # local row 0..31
            wloc = s_ % W
            for k in range(K):
                basec[:, ch, k] = rloc + (k // 3) - 1 + P
                basec[:, ch, 9 + k] = wloc + (k % 3) - 1 + P
        in_maps.append({
            "xstrip": strip,
            "w_t": wt,
            "woff_t": wofft,
            "b_off": boff,
            "bprow": bprow,
            "baseC": basec.reshape(128, 16 * 32),
            "ident32": id32,
            "ident16": id16,
            "perm32": perm,
            "ones16": ones,
        })
    return in_maps


def _get_nc():
    if "nc" not in _NC_CACHE:
        _NC_CACHE["nc"] = build_nc()
    return _NC_CACHE["nc"]


def kernel(**inputs):
    nc = _get_nc()
    in_maps = _host_prep(inputs)
    res = bass_utils.run_bass_kernel_spmd(nc, in_maps, core_ids=list(range(N_CORES)))
    out = np.zeros((B, O, H, W), np.float32)
    for core in range(N_CORES):
        b, half = core // 2, core % 2
        arr = res.results[core]["out_c"].reshape(S, O)  # s = ch*128 + p
        out[b, :, half * HH:(half + 1) * HH, :] = (
            arr.reshape(HH, W, O).transpose(2, 0, 1))
    return out
